# revision 7
# baseline (speedup 1.0000x reference)
"""Trainium2 Bass kernel for ConvQuadInterp3d (3D NMS + quadratic refinement).

Sharding: fully data-parallel. Core c handles plane (b=c//4, d=c%4) of the
(2,1,4,1024,1024) input. Host passes each core its own plane plus the
depth-clamped prev/next planes, so one SPMD program serves all 8 cores.

Per-core kernel (1024x1024 plane, fp32):
  - 9 row-tiles of 126 output rows (+1-row halo top/bottom, edge-replicated
    at volume boundaries during the DMA load; W edge-replication via padded
    [128,1026] tiles).
  - 3x3x3 strict NMS max-pool computed separably (D, then H via partition-
    shifted SBUF->SBUF DMA copies - bit-exact, needed for the x==pooled
    equality mask - then W via shifted views).
  - Hessian terms that cross H go through TensorE as banded shift-matrix
    matmuls (T+1/T-1 combinations) accumulating in PSUM; W/D-only terms are
    plain shifted-view arithmetic on VectorE. hes_noise*EPS is added during
    the PSUM->SBUF evacuation on ScalarE (activation bias), per-partition
    scalars supplied by the host.
  - 3x3 solve via the adjugate/Cramer rule; det is replaced by 1.0 outside
    the NMS mask (memset + copy_predicated) so the reciprocal stays finite.
  - offsets dx = -0.5*sol' (sol' solves with unscaled 2x gradients), with
    the |dx|>0.7 discard folded into one keep multiplier.
"""
import numpy as np
import concourse.bass as bass
import concourse.mybir as mybir
from concourse import tile, bacc
from concourse.bass_utils import run_bass_kernel_spmd

f32 = mybir.dt.float32
Alu = mybir.AluOpType
Act = mybir.ActivationFunctionType

EPS = 1e-7
H = 1024
W = 1024
PW = W + 2
NT = 9           # row tiles
RPT = 126        # valid output rows per tile
CK = 512         # chunk width for the solve phase
NCORES = 8

_CACHE: dict = {}


def _build_nc():
    nc = bacc.Bacc("TRN2", target_bir_lowering=False, debug=False,
                   num_devices=NCORES)
    xp_d = nc.dram_tensor("xp", [H, W], f32, kind="ExternalInput")
    xc_d = nc.dram_tensor("xc", [H, W], f32, kind="ExternalInput")
    xn_d = nc.dram_tensor("xn", [H, W], f32, kind="ExternalInput")
    cst_d = nc.dram_tensor("consts", [128, 16], f32, kind="ExternalInput")
    y_d = nc.dram_tensor("y", [H, W], f32, kind="ExternalOutput")
    cd_d = nc.dram_tensor("cd", [H, W], f32, kind="ExternalOutput")
    ch_d = nc.dram_tensor("ch", [H, W], f32, kind="ExternalOutput")
    cw_d = nc.dram_tensor("cw", [H, W], f32, kind="ExternalOutput")

    # shift matrices (lhsT layout: out[m] = sum_k L[k,m] * in[k])
    Lp = np.zeros((128, 128), np.float32)
    Lp[np.arange(1, 128), np.arange(127)] = 1.0      # out[m] = in[m+1]
    Lm = np.zeros((128, 128), np.float32)
    Lm[np.arange(127), np.arange(1, 128)] = 1.0      # out[m] = in[m-1]
    I = np.eye(128, dtype=np.float32)
    L1_np = Lp - Lm                                  # by' (row grad, x2)
    L2_np = Lp + Lm - 2.0 * I                        # dyy
    L3_np = (0.25 * (Lp - Lm)).astype(np.float32)    # dxy/dys +
    L4_np = (-L3_np).astype(np.float32)              # dxy/dys -
    l1_d = nc.inline_tensor(L1_np, "L1")
    l2_d = nc.inline_tensor(L2_np, "L2")
    l3_d = nc.inline_tensor(L3_np, "L3")
    l4_d = nc.inline_tensor(L4_np, "L4")
    gw_np = np.broadcast_to(np.arange(W, dtype=np.float32), (128, W)).copy()
    gw_d = nc.inline_tensor(gw_np, "gwi")
    pid_np = (np.arange(128, dtype=np.float32) - 1.0).reshape(128, 1).copy()
    pid_d = nc.inline_tensor(pid_np, "pidx")

    V = nc.vector
    G = nc.gpsimd
    S = nc.scalar
    T = nc.tensor

    with tile.TileContext(nc) as tc, \
         tc.tile_pool(name="cst", bufs=1) as cp, \
         tc.tile_pool(name="inp", bufs=2) as ip, \
         tc.tile_pool(name="pool", bufs=1) as pp, \
         tc.tile_pool(name="ck", bufs=1) as kp, \
         tc.tile_pool(name="outp", bufs=2) as op_, \
         tc.tile_pool(name="psum", bufs=2, space="PSUM") as qp:

        L1s = cp.tile([128, 128], f32, tag="L1s")
        L2s = cp.tile([128, 128], f32, tag="L2s")
        L3s = cp.tile([128, 128], f32, tag="L3s")
        L4s = cp.tile([128, 128], f32, tag="L4s")
        gws = cp.tile([128, W], f32, tag="gws")
        cst = cp.tile([128, 16], f32, tag="cstt")
        pid = cp.tile([128, 1], f32, tag="pidt")
        nc.sync.dma_start(out=L1s[:], in_=l1_d[:])
        nc.sync.dma_start(out=L2s[:], in_=l2_d[:])
        nc.sync.dma_start(out=L3s[:], in_=l3_d[:])
        nc.sync.dma_start(out=L4s[:], in_=l4_d[:])
        nc.sync.dma_start(out=gws[:], in_=gw_d[:])
        nc.sync.dma_start(out=cst[:], in_=cst_d[:])
        nc.sync.dma_start(out=pid[:], in_=pid_d[:])

        def nz(i, j):                       # hes_noise[i,j]*EPS per-partition col
            return cst[:, 3 * i + j:3 * i + j + 1]

        dcol = cst[:, 9:10]             # depth value

        for rt in range(NT):
            O = rt * RPT if rt < NT - 1 else H - RPT

            xpt = ip.tile([128, PW], f32, tag="xpt")
            xct = ip.tile([128, PW], f32, tag="xct")
            xnt = ip.tile([128, PW], f32, tag="xnt")
            for t, src in ((xpt, xp_d), (xct, xc_d), (xnt, xn_d)):
                if rt == 0:
                    nc.sync.dma_start(out=t[1:128, 1:1 + W], in_=src[0:127, :])
                    nc.sync.dma_start(out=t[0:1, 1:1 + W], in_=src[0:1, :])
                elif rt == NT - 1:
                    nc.sync.dma_start(out=t[0:127, 1:1 + W],
                                      in_=src[O - 1:O + 126, :])
                    nc.sync.dma_start(out=t[127:128, 1:1 + W],
                                      in_=src[H - 1:H, :])
                else:
                    nc.sync.dma_start(out=t[:, 1:1 + W],
                                      in_=src[O - 1:O + 127, :])
                S.copy(t[:, 0:1], t[:, 1:2])
                S.copy(t[:, PW - 1:PW], t[:, PW - 2:PW - 1])

            # ---- 3x3x3 max pool (separable) ----
            tA = pp.tile([128, PW], f32, tag="tA")
            mD = pp.tile([128, PW], f32, tag="mD")
            V.tensor_tensor(out=tA[:], in0=xpt[:], in1=xct[:], op=Alu.max)
            V.tensor_tensor(out=mD[:], in0=tA[:], in1=xnt[:], op=Alu.max)
            mDu = pp.tile([128, PW], f32, tag="mDu")
            mDd = pp.tile([128, PW], f32, tag="mDd")
            nc.sync.dma_start(out=mDu[0:127, :], in_=mD[1:128, :])
            nc.sync.dma_start(out=mDu[127:128, :], in_=mD[127:128, :])
            nc.sync.dma_start(out=mDd[1:128, :], in_=mD[0:127, :])
            nc.sync.dma_start(out=mDd[0:1, :], in_=mD[0:1, :])
            uH = pp.tile([128, PW], f32, tag="uH")
            mDH = pp.tile([128, PW], f32, tag="mDH")
            V.tensor_tensor(out=uH[:], in0=mDu[:], in1=mD[:], op=Alu.max)
            V.tensor_tensor(out=mDH[:], in0=uH[:], in1=mDd[:], op=Alu.max)
            vW = pp.tile([128, W], f32, tag="vW")
            pooled = pp.tile([128, W], f32, tag="pooled")
            V.tensor_tensor(out=vW[:], in0=mDH[:, 0:W],
                            in1=mDH[:, 1:1 + W], op=Alu.max)
            V.tensor_tensor(out=pooled[:], in0=vW[:],
                            in1=mDH[:, 2:2 + W], op=Alu.max)
            mask = pp.tile([128, W], f32, tag="mask")
            V.tensor_tensor(out=mask[:], in0=xct[:, 1:1 + W],
                            in1=pooled[:], op=Alu.is_equal)

            ght = pp.tile([128, 1], f32, tag="ght")
            V.tensor_scalar(out=ght[:], in0=pid[:], scalar1=float(O),
                            scalar2=None, op0=Alu.add)

            for c in range(2):
                s = c * CK
                r = slice(0, 128)
                XCc = xct[r, 1 + s:1 + s + CK]
                XCp = xct[r, 2 + s:2 + s + CK]
                XCm = xct[r, 0 + s:0 + s + CK]
                XPc = xpt[r, 1 + s:1 + s + CK]
                XPp = xpt[r, 2 + s:2 + s + CK]
                XPm = xpt[r, 0 + s:0 + s + CK]
                XNc = xnt[r, 1 + s:1 + s + CK]
                XNp = xnt[r, 2 + s:2 + s + CK]
                XNm = xnt[r, 0 + s:0 + s + CK]
                maskc = mask[:, s:s + CK]

                # ---- TensorE: H-crossing derivative stencils -> PSUM ----
                byp = qp.tile([128, CK], f32, tag="byp")
                dyyp = qp.tile([128, CK], f32, tag="dyyp")
                dxyp = qp.tile([128, CK], f32, tag="dxyp")
                dysp = qp.tile([128, CK], f32, tag="dysp")
                T.matmul(byp[:], L1s[:], xct[:, 1 + s:1 + s + CK],
                         start=True, stop=True)
                T.matmul(dyyp[:], L2s[:], xct[:, 1 + s:1 + s + CK],
                         start=True, stop=True)
                T.matmul(dxyp[:], L3s[:], xct[:, 2 + s:2 + s + CK],
                         start=True, stop=False)
                T.matmul(dxyp[:], L4s[:], xct[:, 0 + s:0 + s + CK],
                         start=False, stop=True)
                T.matmul(dysp[:], L4s[:], xnt[:, 1 + s:1 + s + CK],
                         start=True, stop=False)
                T.matmul(dysp[:], L3s[:], xpt[:, 1 + s:1 + s + CK],
                         start=False, stop=True)

                # ---- ScalarE: PSUM evacuation with fused noise add ----
                A11 = kp.tile([128, CK], f32, tag="A11")
                A01 = kp.tile([128, CK], f32, tag="A01")
                A10 = kp.tile([128, CK], f32, tag="A10")
                A12 = kp.tile([128, CK], f32, tag="A12")
                A21 = kp.tile([128, CK], f32, tag="A21")
                bys = kp.tile([128, CK], f32, tag="bys")
                S.activation(A11[r, :], dyyp[r, :], Act.Identity, bias=nz(1, 1))
                S.activation(A01[r, :], dxyp[r, :], Act.Identity, bias=nz(0, 1))
                S.activation(A10[r, :], dxyp[r, :], Act.Identity, bias=nz(1, 0))
                S.activation(A12[r, :], dysp[r, :], Act.Identity, bias=nz(1, 2))
                S.activation(A21[r, :], dysp[r, :], Act.Identity, bias=nz(2, 1))
                S.copy(bys[r, :], byp[r, :])

                # ---- VectorE: W/D-only derivatives ----
                bx = kp.tile([128, CK], f32, tag="bx")
                bz = kp.tile([128, CK], f32, tag="bz")
                u1 = kp.tile([128, CK], f32, tag="u1")
                u2 = kp.tile([128, CK], f32, tag="u2")
                A22 = kp.tile([128, CK], f32, tag="A22")
                A00 = kp.tile([128, CK], f32, tag="A00")
                u3 = kp.tile([128, CK], f32, tag="u3")
                u4 = kp.tile([128, CK], f32, tag="u4")
                dxs = kp.tile([128, CK], f32, tag="dxs")
                A02 = kp.tile([128, CK], f32, tag="A02")
                A20 = kp.tile([128, CK], f32, tag="A20")
                V.tensor_tensor(out=bz[r, :], in0=XPc, in1=XNc, op=Alu.subtract)
                V.scalar_tensor_tensor(out=u1[r, :], in0=XPc, scalar=nz(2, 2),
                                       in1=XNc, op0=Alu.add, op1=Alu.add)
                V.scalar_tensor_tensor(out=A22[r, :], in0=XCc, scalar=-2.0,
                                       in1=u1[r, :], op0=Alu.mult, op1=Alu.add)
                V.tensor_tensor(out=bx[r, :], in0=XCp, in1=XCm, op=Alu.subtract)
                V.scalar_tensor_tensor(out=u2[r, :], in0=XCp, scalar=nz(0, 0),
                                       in1=XCm, op0=Alu.add, op1=Alu.add)
                V.scalar_tensor_tensor(out=A00[r, :], in0=XCc, scalar=-2.0,
                                       in1=u2[r, :], op0=Alu.mult, op1=Alu.add)
                V.tensor_tensor(out=u3[r, :], in0=XNm, in1=XNp, op=Alu.subtract)
                V.tensor_tensor(out=u4[r, :], in0=XPm, in1=XPp, op=Alu.subtract)
                V.tensor_tensor(out=dxs[r, :], in0=u3[r, :], in1=u4[r, :],
                                op=Alu.subtract)
                S.activation(A02[r, :], dxs[r, :], Act.Identity,
                             bias=nz(0, 2), scale=0.25)
                S.activation(A20[r, :], dxs[r, :], Act.Identity,
                             bias=nz(2, 0), scale=0.25)

                # ---- adjugate (cofactor transpose), split across V and G ----
                adj = {}
                scr = [kp.tile([128, CK], f32, tag=f"scr{i}", name=f"scr{i}")
                       for i in range(6)]
                terms = [
                    # (key, p, q, c, d) -> adj = p*q - c*d   (engine alternates)
                    ("00", A11, A22, A12, A21),
                    ("01", A02, A21, A01, A22),
                    ("02", A01, A12, A02, A11),
                    ("10", A12, A20, A10, A22),
                    ("11", A00, A22, A02, A20),
                    ("12", A02, A10, A00, A12),
                    ("20", A10, A21, A11, A20),
                    ("21", A01, A20, A00, A21),
                    ("22", A00, A11, A01, A10),
                ]
                for i, (key, p, q, cc, dd) in enumerate(terms):
                    E = V
                    sa = scr[(2 * i) % 6]
                    sb = scr[(2 * i + 1) % 6]
                    a = kp.tile([128, CK], f32, tag=f"adj{key}")
                    E.tensor_tensor(out=sa[r, :], in0=p[r, :], in1=q[r, :],
                                    op=Alu.mult)
                    E.tensor_tensor(out=sb[r, :], in0=cc[r, :], in1=dd[r, :],
                                    op=Alu.mult)
                    E.tensor_tensor(out=a[r, :], in0=sa[r, :], in1=sb[r, :],
                                    op=Alu.subtract)
                    adj[key] = a

                # ---- det, masked reciprocal ----
                d1 = kp.tile([128, CK], f32, tag="d1")
                d2 = kp.tile([128, CK], f32, tag="d2")
                d3 = kp.tile([128, CK], f32, tag="d3")
                det = kp.tile([128, CK], f32, tag="det")
                V.tensor_tensor(out=d1[r, :], in0=A00[r, :], in1=adj["00"][r, :],
                                op=Alu.mult)
                V.tensor_tensor(out=d2[r, :], in0=A01[r, :], in1=adj["10"][r, :],
                                op=Alu.mult)
                V.tensor_tensor(out=d3[r, :], in0=A02[r, :], in1=adj["20"][r, :],
                                op=Alu.mult)
                V.tensor_tensor(out=d1[r, :], in0=d1[r, :], in1=d2[r, :],
                                op=Alu.add)
                V.tensor_tensor(out=det[r, :], in0=d1[r, :], in1=d3[r, :],
                                op=Alu.add)
                ds = kp.tile([128, CK], f32, tag="ds")
                V.memset(ds[r, :], 1.0)
                V.copy_predicated(ds[r, :], maskc.bitcast(mybir.dt.uint32),
                                  det[r, :])
                rdet = kp.tile([128, CK], f32, tag="rdet")
                V.reciprocal(rdet[r, :], ds[r, :])

                # ---- solution: sol_i = (adj_i0*bx + adj_i1*by + adj_i2*bz)*rdet
                sols = []
                for i, key in enumerate(("0", "1", "2")):
                    m1 = scr[0] if i != 0 else scr[3]
                    m2 = scr[1] if i != 0 else scr[4]
                    m3 = scr[2] if i != 0 else scr[5]
                    E1 = V
                    E2 = V
                    E1.tensor_tensor(out=m1[r, :], in0=adj[key + "0"][r, :],
                                     in1=bx[r, :], op=Alu.mult)
                    E2.tensor_tensor(out=m2[r, :], in0=adj[key + "1"][r, :],
                                     in1=bys[r, :], op=Alu.mult)
                    E1.tensor_tensor(out=m3[r, :], in0=adj[key + "2"][r, :],
                                     in1=bz[r, :], op=Alu.mult)
                    E2.tensor_tensor(out=m1[r, :], in0=m1[r, :], in1=m2[r, :],
                                     op=Alu.add)
                    E1.tensor_tensor(out=m1[r, :], in0=m1[r, :], in1=m3[r, :],
                                     op=Alu.add)
                    so = kp.tile([128, CK], f32, tag=f"sol{key}")
                    E2.tensor_tensor(out=so[r, :], in0=m1[r, :], in1=rdet[r, :],
                                     op=Alu.mult)
                    sols.append(so)

                # ---- discard-big + mask fold: keep = -0.5*mask*(|sol|<=1.4) --
                ab0 = kp.tile([128, CK], f32, tag="ab0")
                ab1 = kp.tile([128, CK], f32, tag="ab1")
                ab2 = kp.tile([128, CK], f32, tag="ab2")
                S.activation(ab0[r, :], sols[0][r, :], Act.Abs)
                S.activation(ab1[r, :], sols[1][r, :], Act.Abs)
                S.activation(ab2[r, :], sols[2][r, :], Act.Abs)
                am1 = kp.tile([128, CK], f32, tag="am1")
                am2 = kp.tile([128, CK], f32, tag="am2")
                V.tensor_tensor(out=am1[r, :], in0=ab0[r, :],
                                in1=ab1[r, :], op=Alu.max)
                V.tensor_tensor(out=am2[r, :], in0=am1[r, :], in1=ab2[r, :],
                                op=Alu.max)
                nb = kp.tile([128, CK], f32, tag="nb")
                V.tensor_scalar(out=nb[r, :], in0=am2[r, :], scalar1=1.4,
                                scalar2=None, op0=Alu.is_le)
                keep = kp.tile([128, CK], f32, tag="keep")
                V.scalar_tensor_tensor(out=keep[r, :], in0=nb[r, :], scalar=-0.5,
                                       in1=maskc, op0=Alu.mult, op1=Alu.mult)
                t0 = kp.tile([128, CK], f32, tag="t0")
                t1 = kp.tile([128, CK], f32, tag="t1")
                t2 = kp.tile([128, CK], f32, tag="t2")
                V.tensor_tensor(out=t0[r, :], in0=sols[0][r, :], in1=keep[r, :],
                                op=Alu.mult)
                V.tensor_tensor(out=t1[r, :], in0=sols[1][r, :], in1=keep[r, :],
                                op=Alu.mult)
                V.tensor_tensor(out=t2[r, :], in0=sols[2][r, :], in1=keep[r, :],
                                op=Alu.mult)

                # ---- outputs ----
                cdt = op_.tile([128, CK], f32, tag="cdt")
                cht = op_.tile([128, CK], f32, tag="cht")
                cwt = op_.tile([128, CK], f32, tag="cwt")
                V.tensor_scalar(out=cdt[r, :], in0=t2[r, :], scalar1=dcol,
                                scalar2=None, op0=Alu.add)
                V.tensor_scalar(out=cht[r, :], in0=t1[r, :],
                                scalar1=ght[:, 0:1], scalar2=None,
                                op0=Alu.add)
                V.tensor_tensor(out=cwt[r, :], in0=t0[r, :],
                                in1=gws[r, s:s + CK], op=Alu.add)
                w1 = scr[0]
                w2 = scr[1]
                w3 = scr[2]
                V.tensor_tensor(out=w1[r, :], in0=bx[r, :], in1=t0[r, :],
                                op=Alu.mult)
                V.tensor_tensor(out=w2[r, :], in0=bys[r, :], in1=t1[r, :],
                                op=Alu.mult)
                V.tensor_tensor(out=w3[r, :], in0=bz[r, :], in1=t2[r, :],
                                op=Alu.mult)
                V.tensor_tensor(out=w1[r, :], in0=w1[r, :], in1=w2[r, :],
                                op=Alu.add)
                V.tensor_tensor(out=w1[r, :], in0=w1[r, :], in1=w3[r, :],
                                op=Alu.add)
                y1 = kp.tile([128, CK], f32, tag="y1")
                y2 = op_.tile([128, CK], f32, tag="y2")
                V.scalar_tensor_tensor(out=y1[r, :], in0=w1[r, :], scalar=0.25,
                                       in1=XCc, op0=Alu.mult, op1=Alu.add)
                V.scalar_tensor_tensor(out=y2[r, :], in0=maskc, scalar=10.0,
                                       in1=y1[r, :], op0=Alu.mult, op1=Alu.add)

                nc.sync.dma_start(out=y_d[O:O + RPT, s:s + CK],
                                  in_=y2[1:127, :])
                nc.sync.dma_start(out=cd_d[O:O + RPT, s:s + CK],
                                  in_=cdt[1:127, :])
                nc.sync.dma_start(out=ch_d[O:O + RPT, s:s + CK],
                                  in_=cht[1:127, :])
                nc.sync.dma_start(out=cw_d[O:O + RPT, s:s + CK],
                                  in_=cwt[1:127, :])

    nc.compile()
    return nc


def _get_nc():
    if "nc" not in _CACHE:
        _CACHE["nc"] = _build_nc()
    return _CACHE["nc"]


def make_core_inputs(x, hes_noise):
    """Host-side sharding: per-core input dicts."""
    x = np.asarray(x)
    hes_noise = np.asarray(hes_noise, dtype=np.float32)
    B, C, D = x.shape[0], x.shape[1], x.shape[2]
    ins = []
    for c in range(NCORES):
        b, d = c // D, c % D
        cst = np.zeros((128, 16), np.float32)
        cst[:, 0:9] = (hes_noise.reshape(-1) * np.float32(EPS))[None, :]
        cst[:, 9] = np.float32(d)
        ins.append({
            "xp": np.ascontiguousarray(x[b, 0, max(d - 1, 0)]),
            "xc": np.ascontiguousarray(x[b, 0, d]),
            "xn": np.ascontiguousarray(x[b, 0, min(d + 1, D - 1)]),
            "consts": cst,
        })
    return ins


def assemble_outputs(results, B=2, C=1, D=4):
    coords = np.empty((B, C, 3, D, H, W), np.float32)
    y = np.empty((B, C, D, H, W), np.float32)
    for c in range(NCORES):
        b, d = c // D, c % D
        coords[b, 0, 0, d] = results[c]["cd"]
        coords[b, 0, 1, d] = results[c]["ch"]
        coords[b, 0, 2, d] = results[c]["cw"]
        y[b, 0, d] = results[c]["y"]
    return coords, y


def kernel(x, hes_noise):
    nc = _get_nc()
    ins = make_core_inputs(x, hes_noise)
    res = run_bass_kernel_spmd(nc, ins, core_ids=list(range(NCORES)))
    return assemble_outputs(res.results)


# revision 10
# speedup vs baseline: 17.7467x; 17.7467x over previous
"""Trainium2 Bass kernel for ConvQuadInterp3d (3D NMS + quadratic refinement).

Sharding: fully data-parallel. Core c handles plane (b=c//4, d=c%4) of the
(2,1,4,1024,1024) input. Host passes each core its own plane plus the
depth-clamped prev/next planes, so one SPMD program serves all 8 cores.

Per-core kernel (1024x1024 plane, fp32):
  - 9 row-tiles of 126 output rows (+1-row halo top/bottom, edge-replicated
    at volume boundaries during the DMA load; W edge-replication via padded
    [128,1026] tiles).
  - 3x3x3 strict NMS max-pool computed separably (D, then H via partition-
    shifted SBUF->SBUF DMA copies - bit-exact, needed for the x==pooled
    equality mask - then W via shifted views).
  - Hessian terms that cross H go through TensorE as banded shift-matrix
    matmuls (T+1/T-1 combinations) accumulating in PSUM; W/D-only terms are
    plain shifted-view arithmetic on VectorE. hes_noise*EPS is added during
    the PSUM->SBUF evacuation on ScalarE (activation bias), per-partition
    scalars supplied by the host.
  - 3x3 solve via the adjugate/Cramer rule; det is replaced by 1.0 outside
    the NMS mask (memset + copy_predicated) so the reciprocal stays finite.
  - offsets dx = -0.5*sol' (sol' solves with unscaled 2x gradients), with
    the |dx|>0.7 discard folded into one keep multiplier.
"""
import numpy as np
import concourse.bass as bass
import concourse.mybir as mybir
from concourse import tile, bacc
from concourse.bass_utils import run_bass_kernel_spmd

f32 = mybir.dt.float32
Alu = mybir.AluOpType
Act = mybir.ActivationFunctionType

EPS = 1e-7
H = 1024
W = 1024
PW = W + 2
NT = 9           # row tiles
RPT = 126        # valid output rows per tile
CK = 512         # chunk width for the solve phase
NCORES = 8

_CACHE: dict = {}


def _build_nc(reps=1):
    nc = bacc.Bacc("TRN2", target_bir_lowering=False, debug=False,
                   num_devices=NCORES)
    xp_d = nc.dram_tensor("xp", [H, W], f32, kind="ExternalInput")
    xc_d = nc.dram_tensor("xc", [H, W], f32, kind="ExternalInput")
    xn_d = nc.dram_tensor("xn", [H, W], f32, kind="ExternalInput")
    cst_d = nc.dram_tensor("consts", [128, 16], f32, kind="ExternalInput")
    y_d = nc.dram_tensor("y", [H, W], f32, kind="ExternalOutput")
    cd_d = nc.dram_tensor("cd", [H, W], f32, kind="ExternalOutput")
    ch_d = nc.dram_tensor("ch", [H, W], f32, kind="ExternalOutput")
    cw_d = nc.dram_tensor("cw", [H, W], f32, kind="ExternalOutput")

    # shift matrices (lhsT layout: out[m] = sum_k L[k,m] * in[k])
    Lp = np.zeros((128, 128), np.float32)
    Lp[np.arange(1, 128), np.arange(127)] = 1.0      # out[m] = in[m+1]
    Lm = np.zeros((128, 128), np.float32)
    Lm[np.arange(127), np.arange(1, 128)] = 1.0      # out[m] = in[m-1]
    I = np.eye(128, dtype=np.float32)
    L1_np = Lp - Lm                                  # by' (row grad, x2)
    L2_np = Lp + Lm - 2.0 * I                        # dyy
    L3_np = (0.25 * (Lp - Lm)).astype(np.float32)    # dxy/dys +
    L4_np = (-L3_np).astype(np.float32)              # dxy/dys -
    Ieye_np = I.copy()
    In2_np = (-2.0 * I).astype(np.float32)
    Ip025_np = (0.25 * I).astype(np.float32)
    In025_np = (-0.25 * I).astype(np.float32)
    l1_d = nc.inline_tensor(L1_np, "L1")
    l2_d = nc.inline_tensor(L2_np, "L2")
    l3_d = nc.inline_tensor(L3_np, "L3")
    l4_d = nc.inline_tensor(L4_np, "L4")
    gw_np = np.broadcast_to(np.arange(W, dtype=np.float32), (128, W)).copy()
    gw_d = nc.inline_tensor(gw_np, "gwi")
    pid_np = (np.arange(128, dtype=np.float32) - 1.0).reshape(128, 1).copy()
    pid_d = nc.inline_tensor(pid_np, "pidx")
    ie_d = nc.inline_tensor(Ieye_np, "Ieye")
    in2_d = nc.inline_tensor(In2_np, "In2")
    ip_d = nc.inline_tensor(Ip025_np, "Ip025")
    in_d = nc.inline_tensor(In025_np, "In025")

    V = nc.vector
    G = nc.gpsimd
    S = nc.scalar
    T = nc.tensor

    with tile.TileContext(nc) as tc, \
         tc.tile_pool(name="cst", bufs=1) as cp, \
         tc.tile_pool(name="inp", bufs=2) as ip, \
         tc.tile_pool(name="pool", bufs=1) as pp, \
         tc.tile_pool(name="ck", bufs=1) as kp, \
         tc.tile_pool(name="outp", bufs=2) as op_, \
         tc.tile_pool(name="psum", bufs=1, space="PSUM") as qp:

        L1s = cp.tile([128, 128], f32, tag="L1s")
        L2s = cp.tile([128, 128], f32, tag="L2s")
        L3s = cp.tile([128, 128], f32, tag="L3s")
        L4s = cp.tile([128, 128], f32, tag="L4s")
        gws = cp.tile([128, W], f32, tag="gws")
        cst = cp.tile([128, 16], f32, tag="cstt")
        pid = cp.tile([128, 1], f32, tag="pidt")
        Ies = cp.tile([128, 128], f32, tag="Ies")
        In2s = cp.tile([128, 128], f32, tag="In2s")
        Ips = cp.tile([128, 128], f32, tag="Ips")
        Ins = cp.tile([128, 128], f32, tag="Ins")
        nc.sync.dma_start(out=L1s[:], in_=l1_d[:])
        nc.sync.dma_start(out=L2s[:], in_=l2_d[:])
        nc.sync.dma_start(out=L3s[:], in_=l3_d[:])
        nc.sync.dma_start(out=L4s[:], in_=l4_d[:])
        nc.sync.dma_start(out=gws[:], in_=gw_d[:])
        nc.sync.dma_start(out=cst[:], in_=cst_d[:])
        nc.sync.dma_start(out=pid[:], in_=pid_d[:])
        nc.sync.dma_start(out=Ies[:], in_=ie_d[:])
        nc.sync.dma_start(out=In2s[:], in_=in2_d[:])
        nc.sync.dma_start(out=Ips[:], in_=ip_d[:])
        nc.sync.dma_start(out=Ins[:], in_=in_d[:])

        def nz(i, j):                       # hes_noise[i,j]*EPS per-partition col
            return cst[:, 3 * i + j:3 * i + j + 1]

        dcol = cst[:, 9:10]             # depth value

        for rep in range(reps):
         for rt in range(NT):
            O = rt * RPT if rt < NT - 1 else H - RPT

            xpt = ip.tile([128, PW], f32, tag="xpt")
            xct = ip.tile([128, PW], f32, tag="xct")
            xnt = ip.tile([128, PW], f32, tag="xnt")
            for t, src in ((xpt, xp_d), (xct, xc_d), (xnt, xn_d)):
                if rt == 0:
                    nc.sync.dma_start(out=t[1:128, 1:1 + W], in_=src[0:127, :])
                    nc.sync.dma_start(out=t[0:1, 1:1 + W], in_=src[0:1, :])
                elif rt == NT - 1:
                    nc.sync.dma_start(out=t[0:127, 1:1 + W],
                                      in_=src[O - 1:O + 126, :])
                    nc.sync.dma_start(out=t[127:128, 1:1 + W],
                                      in_=src[H - 1:H, :])
                else:
                    nc.sync.dma_start(out=t[:, 1:1 + W],
                                      in_=src[O - 1:O + 127, :])
                S.copy(t[:, 0:1], t[:, 1:2])
                S.copy(t[:, PW - 1:PW], t[:, PW - 2:PW - 1])

            # ---- 3x3x3 max pool (separable) ----
            tA = pp.tile([128, PW], f32, tag="tA")
            mD = pp.tile([128, PW], f32, tag="mD")
            V.tensor_tensor(out=tA[:], in0=xpt[:], in1=xct[:], op=Alu.max)
            V.tensor_tensor(out=mD[:], in0=tA[:], in1=xnt[:], op=Alu.max)
            mDu = pp.tile([128, PW], f32, tag="mDu")
            mDd = pp.tile([128, PW], f32, tag="mDd")
            nc.sync.dma_start(out=mDu[0:127, :], in_=mD[1:128, :])
            nc.sync.dma_start(out=mDu[127:128, :], in_=mD[127:128, :])
            nc.sync.dma_start(out=mDd[1:128, :], in_=mD[0:127, :])
            nc.sync.dma_start(out=mDd[0:1, :], in_=mD[0:1, :])
            uH = pp.tile([128, PW], f32, tag="uH")
            mDH = pp.tile([128, PW], f32, tag="mDH")
            V.tensor_tensor(out=uH[:], in0=mDu[:], in1=mD[:], op=Alu.max)
            V.tensor_tensor(out=mDH[:], in0=uH[:], in1=mDd[:], op=Alu.max)
            vW = pp.tile([128, W], f32, tag="vW")
            pooled = pp.tile([128, W], f32, tag="pooled")
            V.tensor_tensor(out=vW[:], in0=mDH[:, 0:W],
                            in1=mDH[:, 1:1 + W], op=Alu.max)
            V.tensor_tensor(out=pooled[:], in0=vW[:],
                            in1=mDH[:, 2:2 + W], op=Alu.max)
            mask = pp.tile([128, W], f32, tag="mask")
            V.tensor_tensor(out=mask[:], in0=xct[:, 1:1 + W],
                            in1=pooled[:], op=Alu.is_equal)

            ght = pp.tile([128, 1], f32, tag="ght")
            V.tensor_scalar(out=ght[:], in0=pid[:], scalar1=float(O),
                            scalar2=None, op0=Alu.add)

            for c in range(2):
                s = c * CK
                r = slice(0, 128)
                XCc = xct[r, 1 + s:1 + s + CK]
                XCp = xct[r, 2 + s:2 + s + CK]
                XCm = xct[r, 0 + s:0 + s + CK]
                XPc = xpt[r, 1 + s:1 + s + CK]
                XPp = xpt[r, 2 + s:2 + s + CK]
                XPm = xpt[r, 0 + s:0 + s + CK]
                XNc = xnt[r, 1 + s:1 + s + CK]
                XNp = xnt[r, 2 + s:2 + s + CK]
                XNm = xnt[r, 0 + s:0 + s + CK]
                maskc = mask[:, s:s + CK]

                # ---- TensorE: H-crossing derivative stencils -> PSUM ----
                byp = qp.tile([128, CK], f32, tag="byp")
                dyyp = qp.tile([128, CK], f32, tag="dyyp")
                dxyp = qp.tile([128, CK], f32, tag="dxyp")
                dysp = qp.tile([128, CK], f32, tag="dysp")
                T.matmul(byp[:], L1s[:], xct[:, 1 + s:1 + s + CK],
                         start=True, stop=True)
                T.matmul(dyyp[:], L2s[:], xct[:, 1 + s:1 + s + CK],
                         start=True, stop=True)
                T.matmul(dxyp[:], L3s[:], xct[:, 2 + s:2 + s + CK],
                         start=True, stop=False)
                T.matmul(dxyp[:], L4s[:], xct[:, 0 + s:0 + s + CK],
                         start=False, stop=True)
                T.matmul(dysp[:], L4s[:], xnt[:, 1 + s:1 + s + CK],
                         start=True, stop=False)
                T.matmul(dysp[:], L3s[:], xpt[:, 1 + s:1 + s + CK],
                         start=False, stop=True)
                dxxp = qp.tile([128, CK], f32, tag="dxxp")
                dssp = qp.tile([128, CK], f32, tag="dssp")
                dxsp = qp.tile([128, CK], f32, tag="dxsp")
                T.matmul(dxxp[:], Ies[:], xct[:, 2 + s:2 + s + CK],
                         start=True, stop=False)
                T.matmul(dxxp[:], Ies[:], xct[:, 0 + s:0 + s + CK],
                         start=False, stop=False)
                T.matmul(dxxp[:], In2s[:], xct[:, 1 + s:1 + s + CK],
                         start=False, stop=True)
                T.matmul(dssp[:], Ies[:], xpt[:, 1 + s:1 + s + CK],
                         start=True, stop=False)
                T.matmul(dssp[:], Ies[:], xnt[:, 1 + s:1 + s + CK],
                         start=False, stop=False)
                T.matmul(dssp[:], In2s[:], xct[:, 1 + s:1 + s + CK],
                         start=False, stop=True)
                T.matmul(dxsp[:], Ips[:], xnt[:, 0 + s:0 + s + CK],
                         start=True, stop=False)
                T.matmul(dxsp[:], Ins[:], xnt[:, 2 + s:2 + s + CK],
                         start=False, stop=False)
                T.matmul(dxsp[:], Ins[:], xpt[:, 0 + s:0 + s + CK],
                         start=False, stop=False)
                T.matmul(dxsp[:], Ips[:], xpt[:, 2 + s:2 + s + CK],
                         start=False, stop=True)

                # ---- ScalarE: PSUM evacuation with fused noise add ----
                A11 = kp.tile([128, CK], f32, tag="A11")
                A01 = kp.tile([128, CK], f32, tag="A01")
                A10 = kp.tile([128, CK], f32, tag="A10")
                A12 = kp.tile([128, CK], f32, tag="A12")
                A21 = kp.tile([128, CK], f32, tag="A21")
                bys = kp.tile([128, CK], f32, tag="bys")
                S.activation(A11[r, :], dyyp[r, :], Act.Identity, bias=nz(1, 1))
                S.activation(A01[r, :], dxyp[r, :], Act.Identity, bias=nz(0, 1))
                S.activation(A10[r, :], dxyp[r, :], Act.Identity, bias=nz(1, 0))
                S.activation(A12[r, :], dysp[r, :], Act.Identity, bias=nz(1, 2))
                S.activation(A21[r, :], dysp[r, :], Act.Identity, bias=nz(2, 1))
                S.copy(bys[r, :], byp[r, :])
                A00 = kp.tile([128, CK], f32, tag="A00")
                A22 = kp.tile([128, CK], f32, tag="A22")
                A02 = kp.tile([128, CK], f32, tag="A02")
                A20 = kp.tile([128, CK], f32, tag="A20")
                S.activation(A00[r, :], dxxp[r, :], Act.Identity, bias=nz(0, 0))
                S.activation(A22[r, :], dssp[r, :], Act.Identity, bias=nz(2, 2))
                S.activation(A02[r, :], dxsp[r, :], Act.Identity, bias=nz(0, 2))
                S.activation(A20[r, :], dxsp[r, :], Act.Identity, bias=nz(2, 0))

                # ---- VectorE: W/D-only derivatives ----
                bx = kp.tile([128, CK], f32, tag="bx")
                bz = kp.tile([128, CK], f32, tag="bz")
                V.tensor_tensor(out=bz[r, :], in0=XPc, in1=XNc, op=Alu.subtract)
                V.tensor_tensor(out=bx[r, :], in0=XCp, in1=XCm, op=Alu.subtract)

                # ---- adjugate (cofactor transpose), split across V and G ----
                adj = {}
                scr = [kp.tile([128, CK], f32, tag=f"scr{i}", name=f"scr{i}")
                       for i in range(6)]
                terms = [
                    # (key, p, q, c, d) -> adj = p*q - c*d   (engine alternates)
                    ("00", A11, A22, A12, A21),
                    ("01", A02, A21, A01, A22),
                    ("02", A01, A12, A02, A11),
                    ("10", A12, A20, A10, A22),
                    ("11", A00, A22, A02, A20),
                    ("12", A02, A10, A00, A12),
                    ("20", A10, A21, A11, A20),
                    ("21", A01, A20, A00, A21),
                    ("22", A00, A11, A01, A10),
                ]
                for i, (key, p, q, cc, dd) in enumerate(terms):
                    E = V
                    sa = scr[(2 * i) % 6]
                    sb = scr[(2 * i + 1) % 6]
                    a = kp.tile([128, CK], f32, tag=f"adj{key}")
                    E.tensor_tensor(out=sa[r, :], in0=p[r, :], in1=q[r, :],
                                    op=Alu.mult)
                    E.tensor_tensor(out=sb[r, :], in0=cc[r, :], in1=dd[r, :],
                                    op=Alu.mult)
                    E.tensor_tensor(out=a[r, :], in0=sa[r, :], in1=sb[r, :],
                                    op=Alu.subtract)
                    adj[key] = a

                # ---- det, masked reciprocal ----
                d1 = kp.tile([128, CK], f32, tag="d1")
                d2 = kp.tile([128, CK], f32, tag="d2")
                d3 = kp.tile([128, CK], f32, tag="d3")
                det = kp.tile([128, CK], f32, tag="det")
                V.tensor_tensor(out=d1[r, :], in0=A00[r, :], in1=adj["00"][r, :],
                                op=Alu.mult)
                V.tensor_tensor(out=d2[r, :], in0=A01[r, :], in1=adj["10"][r, :],
                                op=Alu.mult)
                V.tensor_tensor(out=d3[r, :], in0=A02[r, :], in1=adj["20"][r, :],
                                op=Alu.mult)
                V.tensor_tensor(out=d1[r, :], in0=d1[r, :], in1=d2[r, :],
                                op=Alu.add)
                V.tensor_tensor(out=det[r, :], in0=d1[r, :], in1=d3[r, :],
                                op=Alu.add)
                ds = kp.tile([128, CK], f32, tag="ds")
                G.memset(ds[r, :], 1.0)
                V.copy_predicated(ds[r, :], maskc.bitcast(mybir.dt.uint32),
                                  det[r, :])
                rdet = kp.tile([128, CK], f32, tag="rdet")
                V.reciprocal(rdet[r, :], ds[r, :])

                # ---- solution: sol_i = (adj_i0*bx + adj_i1*by + adj_i2*bz)*rdet
                sols = []
                for i, key in enumerate(("0", "1", "2")):
                    m1 = scr[0] if i != 0 else scr[3]
                    m2 = scr[1] if i != 0 else scr[4]
                    m3 = scr[2] if i != 0 else scr[5]
                    E1 = V
                    E2 = V
                    E1.tensor_tensor(out=m1[r, :], in0=adj[key + "0"][r, :],
                                     in1=bx[r, :], op=Alu.mult)
                    E2.tensor_tensor(out=m2[r, :], in0=adj[key + "1"][r, :],
                                     in1=bys[r, :], op=Alu.mult)
                    E1.tensor_tensor(out=m3[r, :], in0=adj[key + "2"][r, :],
                                     in1=bz[r, :], op=Alu.mult)
                    E2.tensor_tensor(out=m1[r, :], in0=m1[r, :], in1=m2[r, :],
                                     op=Alu.add)
                    E1.tensor_tensor(out=m1[r, :], in0=m1[r, :], in1=m3[r, :],
                                     op=Alu.add)
                    so = kp.tile([128, CK], f32, tag=f"sol{key}")
                    E2.tensor_tensor(out=so[r, :], in0=m1[r, :], in1=rdet[r, :],
                                     op=Alu.mult)
                    sols.append(so)

                # ---- discard-big + mask fold: keep = -0.5*mask*(|sol|<=1.4) --
                ab0 = kp.tile([128, CK], f32, tag="ab0")
                ab1 = kp.tile([128, CK], f32, tag="ab1")
                ab2 = kp.tile([128, CK], f32, tag="ab2")
                S.activation(ab0[r, :], sols[0][r, :], Act.Abs)
                S.activation(ab1[r, :], sols[1][r, :], Act.Abs)
                S.activation(ab2[r, :], sols[2][r, :], Act.Abs)
                am1 = kp.tile([128, CK], f32, tag="am1")
                am2 = kp.tile([128, CK], f32, tag="am2")
                V.tensor_tensor(out=am1[r, :], in0=ab0[r, :],
                                in1=ab1[r, :], op=Alu.max)
                V.tensor_tensor(out=am2[r, :], in0=am1[r, :], in1=ab2[r, :],
                                op=Alu.max)
                nb = kp.tile([128, CK], f32, tag="nb")
                G.tensor_scalar(out=nb[r, :], in0=am2[r, :], scalar1=1.4,
                                scalar2=-0.5, op0=Alu.is_le, op1=Alu.mult)
                keep = kp.tile([128, CK], f32, tag="keep")
                V.tensor_tensor(out=keep[r, :], in0=nb[r, :], in1=maskc,
                                op=Alu.mult)
                t0 = kp.tile([128, CK], f32, tag="t0")
                t1 = kp.tile([128, CK], f32, tag="t1")
                t2 = kp.tile([128, CK], f32, tag="t2")
                V.tensor_tensor(out=t0[r, :], in0=sols[0][r, :], in1=keep[r, :],
                                op=Alu.mult)
                V.tensor_tensor(out=t1[r, :], in0=sols[1][r, :], in1=keep[r, :],
                                op=Alu.mult)
                V.tensor_tensor(out=t2[r, :], in0=sols[2][r, :], in1=keep[r, :],
                                op=Alu.mult)

                # ---- outputs ----
                cdt = op_.tile([128, CK], f32, tag="cdt")
                cht = op_.tile([128, CK], f32, tag="cht")
                cwt = op_.tile([128, CK], f32, tag="cwt")
                G.tensor_scalar(out=cdt[r, :], in0=t2[r, :], scalar1=dcol,
                                scalar2=None, op0=Alu.add)
                G.tensor_scalar(out=cht[r, :], in0=t1[r, :],
                                scalar1=ght[:, 0:1], scalar2=None,
                                op0=Alu.add)
                V.tensor_tensor(out=cwt[r, :], in0=t0[r, :],
                                in1=gws[r, s:s + CK], op=Alu.add)
                w1 = scr[0]
                w2 = scr[1]
                w3 = scr[2]
                V.tensor_tensor(out=w1[r, :], in0=bx[r, :], in1=t0[r, :],
                                op=Alu.mult)
                V.tensor_tensor(out=w2[r, :], in0=bys[r, :], in1=t1[r, :],
                                op=Alu.mult)
                V.tensor_tensor(out=w3[r, :], in0=bz[r, :], in1=t2[r, :],
                                op=Alu.mult)
                V.tensor_tensor(out=w1[r, :], in0=w1[r, :], in1=w2[r, :],
                                op=Alu.add)
                V.tensor_tensor(out=w1[r, :], in0=w1[r, :], in1=w3[r, :],
                                op=Alu.add)
                y1 = kp.tile([128, CK], f32, tag="y1")
                y2 = op_.tile([128, CK], f32, tag="y2")
                V.scalar_tensor_tensor(out=y1[r, :], in0=w1[r, :], scalar=0.25,
                                       in1=XCc, op0=Alu.mult, op1=Alu.add)
                V.scalar_tensor_tensor(out=y2[r, :], in0=maskc, scalar=10.0,
                                       in1=y1[r, :], op0=Alu.mult, op1=Alu.add)

                nc.sync.dma_start(out=y_d[O:O + RPT, s:s + CK],
                                  in_=y2[1:127, :])
                nc.sync.dma_start(out=cd_d[O:O + RPT, s:s + CK],
                                  in_=cdt[1:127, :])
                nc.sync.dma_start(out=ch_d[O:O + RPT, s:s + CK],
                                  in_=cht[1:127, :])
                nc.sync.dma_start(out=cw_d[O:O + RPT, s:s + CK],
                                  in_=cwt[1:127, :])

    nc.compile()
    return nc


def _get_nc(reps=1):
    key = f"nc{reps}"
    if key not in _CACHE:
        _CACHE[key] = _build_nc(reps)
    return _CACHE[key]


def make_core_inputs(x, hes_noise):
    """Host-side sharding: per-core input dicts."""
    x = np.asarray(x)
    hes_noise = np.asarray(hes_noise, dtype=np.float32)
    B, C, D = x.shape[0], x.shape[1], x.shape[2]
    ins = []
    for c in range(NCORES):
        b, d = c // D, c % D
        cst = np.zeros((128, 16), np.float32)
        cst[:, 0:9] = (hes_noise.reshape(-1) * np.float32(EPS))[None, :]
        cst[:, 9] = np.float32(d)
        ins.append({
            "xp": np.ascontiguousarray(x[b, 0, max(d - 1, 0)]),
            "xc": np.ascontiguousarray(x[b, 0, d]),
            "xn": np.ascontiguousarray(x[b, 0, min(d + 1, D - 1)]),
            "consts": cst,
        })
    return ins


def assemble_outputs(results, B=2, C=1, D=4):
    coords = np.empty((B, C, 3, D, H, W), np.float32)
    y = np.empty((B, C, D, H, W), np.float32)
    for c in range(NCORES):
        b, d = c // D, c % D
        coords[b, 0, 0, d] = results[c]["cd"]
        coords[b, 0, 1, d] = results[c]["ch"]
        coords[b, 0, 2, d] = results[c]["cw"]
        y[b, 0, d] = results[c]["y"]
    return coords, y


def kernel(x, hes_noise):
    nc = _get_nc()
    ins = make_core_inputs(x, hes_noise)
    res = run_bass_kernel_spmd(nc, ins, core_ids=list(range(NCORES)))
    return assemble_outputs(res.results)


# revision 12
# speedup vs baseline: 22.2493x; 1.2537x over previous
"""Trainium2 Bass kernel for ConvQuadInterp3d (3D NMS + quadratic refinement).

Sharding: fully data-parallel. Core c handles plane (b=c//4, d=c%4) of the
(2,1,4,1024,1024) input. Host passes each core its own plane plus the
depth-clamped prev/next planes, so one SPMD program serves all 8 cores.

Per-core kernel (1024x1024 plane, fp32):
  - 9 row-tiles of 126 output rows (+1-row halo top/bottom, edge-replicated
    at volume boundaries during the DMA load; W edge-replication via padded
    [128,1026] tiles).
  - 3x3x3 strict NMS max-pool computed separably (D, then H via partition-
    shifted SBUF->SBUF DMA copies - bit-exact, needed for the x==pooled
    equality mask - then W via shifted views).
  - Hessian terms that cross H go through TensorE as banded shift-matrix
    matmuls (T+1/T-1 combinations) accumulating in PSUM; W/D-only terms are
    plain shifted-view arithmetic on VectorE. hes_noise*EPS is added during
    the PSUM->SBUF evacuation on ScalarE (activation bias), per-partition
    scalars supplied by the host.
  - 3x3 solve via the adjugate/Cramer rule; det is replaced by 1.0 outside
    the NMS mask (memset + copy_predicated) so the reciprocal stays finite.
  - offsets dx = -0.5*sol' (sol' solves with unscaled 2x gradients), with
    the |dx|>0.7 discard folded into one keep multiplier.

Measured (8-core SPMD, axon trn2): relative error vs reference 6.0e-08;
device time per pass ~1.2-1.35 ms (reps-slope method; wall-clock per call is
dominated by ~20-30 ms of axon-relay I/O streaming). VectorE is the wall:
this walrus build rejects GpSimd TensorTensor (Pool-engine check), so all
2-input elementwise work (cofactors/solve) serializes on DVE with its
per-op pipe-drain; TensorE carries all linear stencils, ScalarE the
PSUM evacuations + noise adds, GpSimd memset/tensor_scalar ops.
"""
import numpy as np
import concourse.bass as bass
import concourse.mybir as mybir
from concourse import tile, bacc
from concourse.bass_utils import run_bass_kernel_spmd

f32 = mybir.dt.float32
Alu = mybir.AluOpType
Act = mybir.ActivationFunctionType

EPS = 1e-7
H = 1024
W = 1024
PW = W + 2
NT = 8           # row tiles
RPT = 126        # valid output rows per tile
CK = 512         # chunk width for the solve phase
NCORES = 8

_CACHE: dict = {}


def _build_nc(reps=1):
    nc = bacc.Bacc("TRN2", target_bir_lowering=False, debug=False,
                   num_devices=NCORES)
    xp_d = nc.dram_tensor("xp", [H, W], f32, kind="ExternalInput")
    xc_d = nc.dram_tensor("xc", [H, W], f32, kind="ExternalInput")
    xn_d = nc.dram_tensor("xn", [H, W], f32, kind="ExternalInput")
    cst_d = nc.dram_tensor("consts", [128, 16], f32, kind="ExternalInput")
    y_d = nc.dram_tensor("y", [H, W], f32, kind="ExternalOutput")
    cd_d = nc.dram_tensor("cd", [H, W], f32, kind="ExternalOutput")
    ch_d = nc.dram_tensor("ch", [H, W], f32, kind="ExternalOutput")
    cw_d = nc.dram_tensor("cw", [H, W], f32, kind="ExternalOutput")

    # shift matrices (lhsT layout: out[m] = sum_k L[k,m] * in[k])
    Lp = np.zeros((128, 128), np.float32)
    Lp[np.arange(1, 128), np.arange(127)] = 1.0      # out[m] = in[m+1]
    Lm = np.zeros((128, 128), np.float32)
    Lm[np.arange(127), np.arange(1, 128)] = 1.0      # out[m] = in[m-1]
    I = np.eye(128, dtype=np.float32)
    L1_np = Lp - Lm                                  # by' (row grad, x2)
    L2_np = Lp + Lm - 2.0 * I                        # dyy
    L3_np = (0.25 * (Lp - Lm)).astype(np.float32)    # dxy/dys +
    L4_np = (-L3_np).astype(np.float32)              # dxy/dys -
    Ieye_np = I.copy()
    In1_np = (-1.0 * I).astype(np.float32)
    In2_np = (-2.0 * I).astype(np.float32)
    Ip025_np = (0.25 * I).astype(np.float32)
    In025_np = (-0.25 * I).astype(np.float32)
    gw_np = np.broadcast_to(np.arange(W, dtype=np.float32), (128, W)).copy()
    gw_d = nc.inline_tensor(gw_np, "gwi")
    pid_np = np.arange(128, dtype=np.float32).reshape(128, 1).copy()
    pid_d = nc.inline_tensor(pid_np, "pidx")
    ie_d = nc.inline_tensor(Ieye_np, "Ieye")
    in1_d = nc.inline_tensor(In1_np, "In1")
    in2_d = nc.inline_tensor(In2_np, "In2")
    ip_d = nc.inline_tensor(Ip025_np, "Ip025")
    in_d = nc.inline_tensor(In025_np, "In025")

    V = nc.vector
    G = nc.gpsimd
    S = nc.scalar
    T = nc.tensor

    with tile.TileContext(nc) as tc, \
         tc.tile_pool(name="cst", bufs=1) as cp, \
         tc.tile_pool(name="inp", bufs=2) as ip, \
         tc.tile_pool(name="pool", bufs=1) as pp, \
         tc.tile_pool(name="ck", bufs=1) as kp, \
         tc.tile_pool(name="outp", bufs=2) as op_, \
         tc.tile_pool(name="psum", bufs=1, space="PSUM") as qp:

        gws = cp.tile([128, W], f32, tag="gws")
        cst = cp.tile([128, 16], f32, tag="cstt")
        pid = cp.tile([128, 1], f32, tag="pidt")
        Ies = cp.tile([128, 128], f32, tag="Ies")
        In1s = cp.tile([128, 128], f32, tag="In1s")
        In2s = cp.tile([128, 128], f32, tag="In2s")
        Ips = cp.tile([128, 128], f32, tag="Ips")
        Ins = cp.tile([128, 128], f32, tag="Ins")
        nc.sync.dma_start(out=gws[:], in_=gw_d[:])
        nc.sync.dma_start(out=cst[:], in_=cst_d[:])
        nc.sync.dma_start(out=pid[:], in_=pid_d[:])
        nc.sync.dma_start(out=Ies[:], in_=ie_d[:])
        nc.sync.dma_start(out=In1s[:], in_=in1_d[:])
        nc.sync.dma_start(out=In2s[:], in_=in2_d[:])
        nc.sync.dma_start(out=Ips[:], in_=ip_d[:])
        nc.sync.dma_start(out=Ins[:], in_=in_d[:])

        def nz(i, j):                       # hes_noise[i,j]*EPS per-partition col
            return cst[:, 3 * i + j:3 * i + j + 1]

        dcol = cst[:, 9:10]             # depth value

        for rep in range(reps):
         for rt in range(NT):
            R = rt * 128

            tiles = {}
            for pl, src_d in (("p", xp_d), ("c", xc_d), ("n", xn_d)):
                for al in ("d", "c", "u"):
                    bufs = 2 if al == "c" else 1
                    t = ip.tile([128, PW], f32, tag=f"x{pl}{al}",
                                name=f"x{pl}{al}", bufs=bufs)
                    if al == "c":
                        nc.sync.dma_start(out=t[:, 1:1 + W],
                                          in_=src_d[R:R + 128, :])
                    elif al == "d":
                        if rt == 0:
                            nc.sync.dma_start(out=t[1:128, 1:1 + W],
                                              in_=src_d[0:127, :])
                            nc.sync.dma_start(out=t[0:1, 1:1 + W],
                                              in_=src_d[0:1, :])
                        else:
                            nc.sync.dma_start(out=t[:, 1:1 + W],
                                              in_=src_d[R - 1:R + 127, :])
                    else:
                        if rt == NT - 1:
                            nc.sync.dma_start(out=t[0:127, 1:1 + W],
                                              in_=src_d[R + 1:H, :])
                            nc.sync.dma_start(out=t[127:128, 1:1 + W],
                                              in_=src_d[H - 1:H, :])
                        else:
                            nc.sync.dma_start(out=t[:, 1:1 + W],
                                              in_=src_d[R + 1:R + 129, :])
                    S.copy(t[:, 0:1], t[:, 1:2])
                    S.copy(t[:, PW - 1:PW], t[:, PW - 2:PW - 1])
                    tiles[pl + al] = t
            xpt, xct, xnt = tiles["pc"], tiles["cc"], tiles["nc"]

            # ---- 3x3x3 max pool (separable, H-first across alignments) ----
            mH = {}
            for pl in ("p", "c", "n"):
                tF = pp.tile([128, PW], f32, tag="tF", name="tF")
                m = pp.tile([128, PW], f32, tag=f"mH{pl}", name=f"mH{pl}")
                V.tensor_tensor(out=tF[:], in0=tiles[pl + "d"][:],
                                in1=tiles[pl + "c"][:], op=Alu.max)
                V.tensor_tensor(out=m[:], in0=tF[:], in1=tiles[pl + "u"][:],
                                op=Alu.max)
                mH[pl] = m
            uD = pp.tile([128, PW], f32, tag="uD")
            mDH = pp.tile([128, PW], f32, tag="mDH")
            V.tensor_tensor(out=uD[:], in0=mH["p"][:], in1=mH["c"][:],
                            op=Alu.max)
            V.tensor_tensor(out=mDH[:], in0=uD[:], in1=mH["n"][:], op=Alu.max)
            vW = pp.tile([128, W], f32, tag="vW")
            pooled = pp.tile([128, W], f32, tag="pooled")
            V.tensor_tensor(out=vW[:], in0=mDH[:, 0:W],
                            in1=mDH[:, 1:1 + W], op=Alu.max)
            V.tensor_tensor(out=pooled[:], in0=vW[:],
                            in1=mDH[:, 2:2 + W], op=Alu.max)
            mask = pp.tile([128, W], f32, tag="mask")
            V.tensor_tensor(out=mask[:], in0=xct[:, 1:1 + W],
                            in1=pooled[:], op=Alu.is_equal)

            ght = pp.tile([128, 1], f32, tag="ght")
            V.tensor_scalar(out=ght[:], in0=pid[:], scalar1=float(R),
                            scalar2=None, op0=Alu.add)

            for c in range(2):
                s = c * CK
                r = slice(0, 128)
                XCc = xct[r, 1 + s:1 + s + CK]
                XCp = xct[r, 2 + s:2 + s + CK]
                XCm = xct[r, 0 + s:0 + s + CK]
                XPc = xpt[r, 1 + s:1 + s + CK]
                XPp = xpt[r, 2 + s:2 + s + CK]
                XPm = xpt[r, 0 + s:0 + s + CK]
                XNc = xnt[r, 1 + s:1 + s + CK]
                XNp = xnt[r, 2 + s:2 + s + CK]
                XNm = xnt[r, 0 + s:0 + s + CK]
                maskc = mask[:, s:s + CK]

                # ---- TensorE: all derivative stencils -> PSUM ----
                xcd, xcu = tiles["cd"], tiles["cu"]
                xpd, xpu = tiles["pd"], tiles["pu"]
                xnd, xnu = tiles["nd"], tiles["nu"]
                byp = qp.tile([128, CK], f32, tag="byp")
                dyyp = qp.tile([128, CK], f32, tag="dyyp")
                dxyp = qp.tile([128, CK], f32, tag="dxyp")
                dysp = qp.tile([128, CK], f32, tag="dysp")
                T.matmul(byp[:], Ies[:], xcu[:, 1 + s:1 + s + CK],
                         start=True, stop=False)
                T.matmul(byp[:], In1s[:], xcd[:, 1 + s:1 + s + CK],
                         start=False, stop=True)
                T.matmul(dyyp[:], Ies[:], xcd[:, 1 + s:1 + s + CK],
                         start=True, stop=False)
                T.matmul(dyyp[:], Ies[:], xcu[:, 1 + s:1 + s + CK],
                         start=False, stop=False)
                T.matmul(dyyp[:], In2s[:], xct[:, 1 + s:1 + s + CK],
                         start=False, stop=True)
                T.matmul(dxyp[:], Ips[:], xcd[:, 0 + s:0 + s + CK],
                         start=True, stop=False)
                T.matmul(dxyp[:], Ins[:], xcd[:, 2 + s:2 + s + CK],
                         start=False, stop=False)
                T.matmul(dxyp[:], Ins[:], xcu[:, 0 + s:0 + s + CK],
                         start=False, stop=False)
                T.matmul(dxyp[:], Ips[:], xcu[:, 2 + s:2 + s + CK],
                         start=False, stop=True)
                T.matmul(dysp[:], Ips[:], xnd[:, 1 + s:1 + s + CK],
                         start=True, stop=False)
                T.matmul(dysp[:], Ins[:], xnu[:, 1 + s:1 + s + CK],
                         start=False, stop=False)
                T.matmul(dysp[:], Ins[:], xpd[:, 1 + s:1 + s + CK],
                         start=False, stop=False)
                T.matmul(dysp[:], Ips[:], xpu[:, 1 + s:1 + s + CK],
                         start=False, stop=True)
                dxxp = qp.tile([128, CK], f32, tag="dxxp")
                dssp = qp.tile([128, CK], f32, tag="dssp")
                dxsp = qp.tile([128, CK], f32, tag="dxsp")
                T.matmul(dxxp[:], Ies[:], xct[:, 2 + s:2 + s + CK],
                         start=True, stop=False)
                T.matmul(dxxp[:], Ies[:], xct[:, 0 + s:0 + s + CK],
                         start=False, stop=False)
                T.matmul(dxxp[:], In2s[:], xct[:, 1 + s:1 + s + CK],
                         start=False, stop=True)
                T.matmul(dssp[:], Ies[:], xpt[:, 1 + s:1 + s + CK],
                         start=True, stop=False)
                T.matmul(dssp[:], Ies[:], xnt[:, 1 + s:1 + s + CK],
                         start=False, stop=False)
                T.matmul(dssp[:], In2s[:], xct[:, 1 + s:1 + s + CK],
                         start=False, stop=True)
                T.matmul(dxsp[:], Ips[:], xnt[:, 0 + s:0 + s + CK],
                         start=True, stop=False)
                T.matmul(dxsp[:], Ins[:], xnt[:, 2 + s:2 + s + CK],
                         start=False, stop=False)
                T.matmul(dxsp[:], Ins[:], xpt[:, 0 + s:0 + s + CK],
                         start=False, stop=False)
                T.matmul(dxsp[:], Ips[:], xpt[:, 2 + s:2 + s + CK],
                         start=False, stop=True)

                # ---- ScalarE: PSUM evacuation with fused noise add ----
                A11 = kp.tile([128, CK], f32, tag="A11")
                A01 = kp.tile([128, CK], f32, tag="A01")
                A10 = kp.tile([128, CK], f32, tag="A10")
                A12 = kp.tile([128, CK], f32, tag="A12")
                A21 = kp.tile([128, CK], f32, tag="A21")
                bys = kp.tile([128, CK], f32, tag="bys")
                S.activation(A11[r, :], dyyp[r, :], Act.Identity, bias=nz(1, 1))
                S.activation(A01[r, :], dxyp[r, :], Act.Identity, bias=nz(0, 1))
                S.activation(A10[r, :], dxyp[r, :], Act.Identity, bias=nz(1, 0))
                S.activation(A12[r, :], dysp[r, :], Act.Identity, bias=nz(1, 2))
                S.activation(A21[r, :], dysp[r, :], Act.Identity, bias=nz(2, 1))
                S.copy(bys[r, :], byp[r, :])
                A00 = kp.tile([128, CK], f32, tag="A00")
                A22 = kp.tile([128, CK], f32, tag="A22")
                A02 = kp.tile([128, CK], f32, tag="A02")
                A20 = kp.tile([128, CK], f32, tag="A20")
                S.activation(A00[r, :], dxxp[r, :], Act.Identity, bias=nz(0, 0))
                S.activation(A22[r, :], dssp[r, :], Act.Identity, bias=nz(2, 2))
                S.activation(A02[r, :], dxsp[r, :], Act.Identity, bias=nz(0, 2))
                S.activation(A20[r, :], dxsp[r, :], Act.Identity, bias=nz(2, 0))

                # ---- VectorE: W/D-only derivatives ----
                bx = kp.tile([128, CK], f32, tag="bx")
                bz = kp.tile([128, CK], f32, tag="bz")
                V.tensor_tensor(out=bz[r, :], in0=XPc, in1=XNc, op=Alu.subtract)
                V.tensor_tensor(out=bx[r, :], in0=XCp, in1=XCm, op=Alu.subtract)

                # ---- adjugate (cofactor transpose), split across V and G ----
                adj = {}
                scr = [kp.tile([128, CK], f32, tag=f"scr{i}", name=f"scr{i}")
                       for i in range(6)]
                terms = [
                    # (key, p, q, c, d) -> adj = p*q - c*d   (engine alternates)
                    ("00", A11, A22, A12, A21),
                    ("01", A02, A21, A01, A22),
                    ("02", A01, A12, A02, A11),
                    ("10", A12, A20, A10, A22),
                    ("11", A00, A22, A02, A20),
                    ("12", A02, A10, A00, A12),
                    ("20", A10, A21, A11, A20),
                    ("21", A01, A20, A00, A21),
                    ("22", A00, A11, A01, A10),
                ]
                for i, (key, p, q, cc, dd) in enumerate(terms):
                    E = V
                    sa = scr[(2 * i) % 6]
                    sb = scr[(2 * i + 1) % 6]
                    a = kp.tile([128, CK], f32, tag=f"adj{key}")
                    E.tensor_tensor(out=sa[r, :], in0=p[r, :], in1=q[r, :],
                                    op=Alu.mult)
                    E.tensor_tensor(out=sb[r, :], in0=cc[r, :], in1=dd[r, :],
                                    op=Alu.mult)
                    E.tensor_tensor(out=a[r, :], in0=sa[r, :], in1=sb[r, :],
                                    op=Alu.subtract)
                    adj[key] = a

                # ---- det, masked reciprocal ----
                d1 = kp.tile([128, CK], f32, tag="d1")
                d2 = kp.tile([128, CK], f32, tag="d2")
                d3 = kp.tile([128, CK], f32, tag="d3")
                det = kp.tile([128, CK], f32, tag="det")
                V.tensor_tensor(out=d1[r, :], in0=A00[r, :], in1=adj["00"][r, :],
                                op=Alu.mult)
                V.tensor_tensor(out=d2[r, :], in0=A01[r, :], in1=adj["10"][r, :],
                                op=Alu.mult)
                V.tensor_tensor(out=d3[r, :], in0=A02[r, :], in1=adj["20"][r, :],
                                op=Alu.mult)
                V.tensor_tensor(out=d1[r, :], in0=d1[r, :], in1=d2[r, :],
                                op=Alu.add)
                V.tensor_tensor(out=det[r, :], in0=d1[r, :], in1=d3[r, :],
                                op=Alu.add)
                ds = kp.tile([128, CK], f32, tag="ds")
                G.memset(ds[r, :], 1.0)
                V.copy_predicated(ds[r, :], maskc.bitcast(mybir.dt.uint32),
                                  det[r, :])
                rdet = kp.tile([128, CK], f32, tag="rdet")
                V.reciprocal(rdet[r, :], ds[r, :])

                # ---- solution: sol_i = (adj_i0*bx + adj_i1*by + adj_i2*bz)*rdet
                sols = []
                for i, key in enumerate(("0", "1", "2")):
                    m1 = scr[0] if i != 0 else scr[3]
                    m2 = scr[1] if i != 0 else scr[4]
                    m3 = scr[2] if i != 0 else scr[5]
                    E1 = V
                    E2 = V
                    E1.tensor_tensor(out=m1[r, :], in0=adj[key + "0"][r, :],
                                     in1=bx[r, :], op=Alu.mult)
                    E2.tensor_tensor(out=m2[r, :], in0=adj[key + "1"][r, :],
                                     in1=bys[r, :], op=Alu.mult)
                    E1.tensor_tensor(out=m3[r, :], in0=adj[key + "2"][r, :],
                                     in1=bz[r, :], op=Alu.mult)
                    E2.tensor_tensor(out=m1[r, :], in0=m1[r, :], in1=m2[r, :],
                                     op=Alu.add)
                    E1.tensor_tensor(out=m1[r, :], in0=m1[r, :], in1=m3[r, :],
                                     op=Alu.add)
                    so = kp.tile([128, CK], f32, tag=f"sol{key}")
                    E2.tensor_tensor(out=so[r, :], in0=m1[r, :], in1=rdet[r, :],
                                     op=Alu.mult)
                    sols.append(so)

                # ---- discard-big + mask fold: keep = -0.5*mask*(|sol|<=1.4) --
                ab0 = kp.tile([128, CK], f32, tag="ab0")
                ab1 = kp.tile([128, CK], f32, tag="ab1")
                ab2 = kp.tile([128, CK], f32, tag="ab2")
                S.activation(ab0[r, :], sols[0][r, :], Act.Abs)
                S.activation(ab1[r, :], sols[1][r, :], Act.Abs)
                S.activation(ab2[r, :], sols[2][r, :], Act.Abs)
                am1 = kp.tile([128, CK], f32, tag="am1")
                am2 = kp.tile([128, CK], f32, tag="am2")
                V.tensor_tensor(out=am1[r, :], in0=ab0[r, :],
                                in1=ab1[r, :], op=Alu.max)
                V.tensor_tensor(out=am2[r, :], in0=am1[r, :], in1=ab2[r, :],
                                op=Alu.max)
                nb = kp.tile([128, CK], f32, tag="nb")
                G.tensor_scalar(out=nb[r, :], in0=am2[r, :], scalar1=1.4,
                                scalar2=-0.5, op0=Alu.is_le, op1=Alu.mult)
                keep = kp.tile([128, CK], f32, tag="keep")
                V.tensor_tensor(out=keep[r, :], in0=nb[r, :], in1=maskc,
                                op=Alu.mult)
                t0 = kp.tile([128, CK], f32, tag="t0")
                t1 = kp.tile([128, CK], f32, tag="t1")
                t2 = kp.tile([128, CK], f32, tag="t2")
                V.tensor_tensor(out=t0[r, :], in0=sols[0][r, :], in1=keep[r, :],
                                op=Alu.mult)
                V.tensor_tensor(out=t1[r, :], in0=sols[1][r, :], in1=keep[r, :],
                                op=Alu.mult)
                V.tensor_tensor(out=t2[r, :], in0=sols[2][r, :], in1=keep[r, :],
                                op=Alu.mult)

                # ---- outputs ----
                cdt = op_.tile([128, CK], f32, tag="cdt")
                cht = op_.tile([128, CK], f32, tag="cht")
                cwt = op_.tile([128, CK], f32, tag="cwt")
                G.tensor_scalar(out=cdt[r, :], in0=t2[r, :], scalar1=dcol,
                                scalar2=None, op0=Alu.add)
                G.tensor_scalar(out=cht[r, :], in0=t1[r, :],
                                scalar1=ght[:, 0:1], scalar2=None,
                                op0=Alu.add)
                V.tensor_tensor(out=cwt[r, :], in0=t0[r, :],
                                in1=gws[r, s:s + CK], op=Alu.add)
                w1 = scr[0]
                w2 = scr[1]
                w3 = scr[2]
                V.tensor_tensor(out=w1[r, :], in0=bx[r, :], in1=t0[r, :],
                                op=Alu.mult)
                V.tensor_tensor(out=w2[r, :], in0=bys[r, :], in1=t1[r, :],
                                op=Alu.mult)
                V.tensor_tensor(out=w3[r, :], in0=bz[r, :], in1=t2[r, :],
                                op=Alu.mult)
                V.tensor_tensor(out=w1[r, :], in0=w1[r, :], in1=w2[r, :],
                                op=Alu.add)
                V.tensor_tensor(out=w1[r, :], in0=w1[r, :], in1=w3[r, :],
                                op=Alu.add)
                y1 = kp.tile([128, CK], f32, tag="y1")
                y2 = op_.tile([128, CK], f32, tag="y2")
                V.scalar_tensor_tensor(out=y1[r, :], in0=w1[r, :], scalar=0.25,
                                       in1=XCc, op0=Alu.mult, op1=Alu.add)
                V.scalar_tensor_tensor(out=y2[r, :], in0=maskc, scalar=10.0,
                                       in1=y1[r, :], op0=Alu.mult, op1=Alu.add)

                nc.sync.dma_start(out=y_d[R:R + 128, s:s + CK], in_=y2[:, :])
                nc.sync.dma_start(out=cd_d[R:R + 128, s:s + CK], in_=cdt[:, :])
                nc.sync.dma_start(out=ch_d[R:R + 128, s:s + CK], in_=cht[:, :])
                nc.sync.dma_start(out=cw_d[R:R + 128, s:s + CK], in_=cwt[:, :])

    nc.compile()
    return nc


def _get_nc(reps=1):
    key = f"nc{reps}"
    if key not in _CACHE:
        _CACHE[key] = _build_nc(reps)
    return _CACHE[key]


def make_core_inputs(x, hes_noise):
    """Host-side sharding: per-core input dicts."""
    x = np.asarray(x)
    hes_noise = np.asarray(hes_noise, dtype=np.float32)
    B, C, D = x.shape[0], x.shape[1], x.shape[2]
    ins = []
    for c in range(NCORES):
        b, d = c // D, c % D
        cst = np.zeros((128, 16), np.float32)
        cst[:, 0:9] = (hes_noise.reshape(-1) * np.float32(EPS))[None, :]
        cst[:, 9] = np.float32(d)
        ins.append({
            "xp": np.ascontiguousarray(x[b, 0, max(d - 1, 0)]),
            "xc": np.ascontiguousarray(x[b, 0, d]),
            "xn": np.ascontiguousarray(x[b, 0, min(d + 1, D - 1)]),
            "consts": cst,
        })
    return ins


def assemble_outputs(results, B=2, C=1, D=4):
    coords = np.empty((B, C, 3, D, H, W), np.float32)
    y = np.empty((B, C, D, H, W), np.float32)
    for c in range(NCORES):
        b, d = c // D, c % D
        coords[b, 0, 0, d] = results[c]["cd"]
        coords[b, 0, 1, d] = results[c]["ch"]
        coords[b, 0, 2, d] = results[c]["cw"]
        y[b, 0, d] = results[c]["y"]
    return coords, y


def kernel(x, hes_noise):
    nc = _get_nc()
    ins = make_core_inputs(x, hes_noise)
    res = run_bass_kernel_spmd(nc, ins, core_ids=list(range(NCORES)))
    return assemble_outputs(res.results)


# revision 13
# speedup vs baseline: 26.0420x; 1.1705x over previous
"""Trainium2 Bass kernel for ConvQuadInterp3d (3D NMS + quadratic refinement).

Sharding: fully data-parallel. Core c handles plane (b=c//4, d=c%4) of the
(2,1,4,1024,1024) input. Host passes each core its own plane plus the
depth-clamped prev/next planes, so one SPMD program serves all 8 cores.

Per-core kernel (1024x1024 plane, fp32):
  - 9 row-tiles of 126 output rows (+1-row halo top/bottom, edge-replicated
    at volume boundaries during the DMA load; W edge-replication via padded
    [128,1026] tiles).
  - 3x3x3 strict NMS max-pool computed separably (D, then H via partition-
    shifted SBUF->SBUF DMA copies - bit-exact, needed for the x==pooled
    equality mask - then W via shifted views).
  - Hessian terms that cross H go through TensorE as banded shift-matrix
    matmuls (T+1/T-1 combinations) accumulating in PSUM; W/D-only terms are
    plain shifted-view arithmetic on VectorE. hes_noise*EPS is added during
    the PSUM->SBUF evacuation on ScalarE (activation bias), per-partition
    scalars supplied by the host.
  - 3x3 solve via the adjugate/Cramer rule; det is replaced by 1.0 outside
    the NMS mask (memset + copy_predicated) so the reciprocal stays finite.
  - offsets dx = -0.5*sol' (sol' solves with unscaled 2x gradients), with
    the |dx|>0.7 discard folded into one keep multiplier.

Measured (8-core SPMD, axon trn2): relative error vs reference 6.0e-08;
device time per pass ~1.2-1.35 ms (reps-slope method; wall-clock per call is
dominated by ~20-30 ms of axon-relay I/O streaming). VectorE is the wall:
this walrus build rejects GpSimd TensorTensor (Pool-engine check), so all
2-input elementwise work (cofactors/solve) serializes on DVE with its
per-op pipe-drain; TensorE carries all linear stencils, ScalarE the
PSUM evacuations + noise adds, GpSimd memset/tensor_scalar ops.
"""
import numpy as np
import concourse.bass as bass
import concourse.mybir as mybir
from concourse import tile, bacc
from concourse.bass_utils import run_bass_kernel_spmd

f32 = mybir.dt.float32
Alu = mybir.AluOpType
Act = mybir.ActivationFunctionType

EPS = 1e-7
H = 1024
W = 1024
PW = W + 2
NT = 8           # row tiles
RPT = 126        # valid output rows per tile
CK = 512         # chunk width for the solve phase
NCORES = 8

_CACHE: dict = {}


def _build_nc(reps=1):
    nc = bacc.Bacc("TRN2", target_bir_lowering=False, debug=False,
                   num_devices=NCORES)
    xp_d = nc.dram_tensor("xp", [H, W], f32, kind="ExternalInput")
    xc_d = nc.dram_tensor("xc", [H, W], f32, kind="ExternalInput")
    xn_d = nc.dram_tensor("xn", [H, W], f32, kind="ExternalInput")
    cst_d = nc.dram_tensor("consts", [128, 16], f32, kind="ExternalInput")
    y_d = nc.dram_tensor("y", [H, W], f32, kind="ExternalOutput")
    cd_d = nc.dram_tensor("cd", [H, W], f32, kind="ExternalOutput")
    ch_d = nc.dram_tensor("ch", [H, W], f32, kind="ExternalOutput")
    cw_d = nc.dram_tensor("cw", [H, W], f32, kind="ExternalOutput")

    # shift matrices (lhsT layout: out[m] = sum_k L[k,m] * in[k])
    Lp = np.zeros((128, 128), np.float32)
    Lp[np.arange(1, 128), np.arange(127)] = 1.0      # out[m] = in[m+1]
    Lm = np.zeros((128, 128), np.float32)
    Lm[np.arange(127), np.arange(1, 128)] = 1.0      # out[m] = in[m-1]
    I = np.eye(128, dtype=np.float32)
    L1_np = Lp - Lm                                  # by' (row grad, x2)
    L2_np = Lp + Lm - 2.0 * I                        # dyy
    L3_np = (0.25 * (Lp - Lm)).astype(np.float32)    # dxy/dys +
    L4_np = (-L3_np).astype(np.float32)              # dxy/dys -
    Ieye_np = I.copy()
    In1_np = (-1.0 * I).astype(np.float32)
    In2_np = (-2.0 * I).astype(np.float32)
    Ip025_np = (0.25 * I).astype(np.float32)
    In025_np = (-0.25 * I).astype(np.float32)
    gw_np = np.broadcast_to(np.arange(W, dtype=np.float32), (128, W)).copy()
    gw_d = nc.inline_tensor(gw_np, "gwi")
    pid_np = np.arange(128, dtype=np.float32).reshape(128, 1).copy()
    pid_d = nc.inline_tensor(pid_np, "pidx")
    ie_d = nc.inline_tensor(Ieye_np, "Ieye")
    in1_d = nc.inline_tensor(In1_np, "In1")
    in2_d = nc.inline_tensor(In2_np, "In2")
    ip_d = nc.inline_tensor(Ip025_np, "Ip025")
    in_d = nc.inline_tensor(In025_np, "In025")

    V = nc.vector
    G = nc.gpsimd
    S = nc.scalar
    T = nc.tensor

    with tile.TileContext(nc) as tc, \
         tc.tile_pool(name="cst", bufs=1) as cp, \
         tc.tile_pool(name="inp", bufs=2) as ip, \
         tc.tile_pool(name="pool", bufs=1) as pp, \
         tc.tile_pool(name="ck", bufs=1) as kp, \
         tc.tile_pool(name="outp", bufs=2) as op_, \
         tc.tile_pool(name="psum", bufs=1, space="PSUM") as qp:

        gws = cp.tile([128, W], f32, tag="gws")
        cst = cp.tile([128, 16], f32, tag="cstt")
        pid = cp.tile([128, 1], f32, tag="pidt")
        Ies = cp.tile([128, 128], f32, tag="Ies")
        In1s = cp.tile([128, 128], f32, tag="In1s")
        In2s = cp.tile([128, 128], f32, tag="In2s")
        Ips = cp.tile([128, 128], f32, tag="Ips")
        Ins = cp.tile([128, 128], f32, tag="Ins")
        nc.sync.dma_start(out=gws[:], in_=gw_d[:])
        nc.sync.dma_start(out=cst[:], in_=cst_d[:])
        nc.sync.dma_start(out=pid[:], in_=pid_d[:])
        nc.sync.dma_start(out=Ies[:], in_=ie_d[:])
        nc.sync.dma_start(out=In1s[:], in_=in1_d[:])
        nc.sync.dma_start(out=In2s[:], in_=in2_d[:])
        nc.sync.dma_start(out=Ips[:], in_=ip_d[:])
        nc.sync.dma_start(out=Ins[:], in_=in_d[:])

        def nz(i, j):                       # hes_noise[i,j]*EPS per-partition col
            return cst[:, 3 * i + j:3 * i + j + 1]

        dcol = cst[:, 9:10]             # depth value

        for rep in range(reps):
         for rt in range(NT):
            R = rt * 128

            tiles = {}
            for pl, src_d in (("p", xp_d), ("c", xc_d), ("n", xn_d)):
                for al in ("d", "c", "u"):
                    bufs = 2 if al == "c" else 1
                    t = ip.tile([128, PW], f32, tag=f"x{pl}{al}",
                                name=f"x{pl}{al}", bufs=bufs)
                    if al == "c":
                        nc.sync.dma_start(out=t[:, 1:1 + W],
                                          in_=src_d[R:R + 128, :])
                    elif al == "d":
                        if rt == 0:
                            nc.sync.dma_start(out=t[1:128, 1:1 + W],
                                              in_=src_d[0:127, :])
                            nc.sync.dma_start(out=t[0:1, 1:1 + W],
                                              in_=src_d[0:1, :])
                        else:
                            nc.sync.dma_start(out=t[:, 1:1 + W],
                                              in_=src_d[R - 1:R + 127, :])
                    else:
                        if rt == NT - 1:
                            nc.sync.dma_start(out=t[0:127, 1:1 + W],
                                              in_=src_d[R + 1:H, :])
                            nc.sync.dma_start(out=t[127:128, 1:1 + W],
                                              in_=src_d[H - 1:H, :])
                        else:
                            nc.sync.dma_start(out=t[:, 1:1 + W],
                                              in_=src_d[R + 1:R + 129, :])
                    G.tensor_copy(out=t[:, 0:1], in_=t[:, 1:2])
                    G.tensor_copy(out=t[:, PW - 1:PW],
                                  in_=t[:, PW - 2:PW - 1])
                    tiles[pl + al] = t
            xpt, xct, xnt = tiles["pc"], tiles["cc"], tiles["nc"]

            # ---- 3x3x3 max pool (separable, H-first across alignments) ----
            mH = {}
            for pl in ("p", "c", "n"):
                tF = pp.tile([128, PW], f32, tag="tF", name="tF")
                m = pp.tile([128, PW], f32, tag=f"mH{pl}", name=f"mH{pl}")
                V.tensor_tensor(out=tF[:], in0=tiles[pl + "d"][:],
                                in1=tiles[pl + "c"][:], op=Alu.max)
                V.tensor_tensor(out=m[:], in0=tF[:], in1=tiles[pl + "u"][:],
                                op=Alu.max)
                mH[pl] = m
            uD = pp.tile([128, PW], f32, tag="uD")
            mDH = pp.tile([128, PW], f32, tag="mDH")
            V.tensor_tensor(out=uD[:], in0=mH["p"][:], in1=mH["c"][:],
                            op=Alu.max)
            V.tensor_tensor(out=mDH[:], in0=uD[:], in1=mH["n"][:], op=Alu.max)
            vW = pp.tile([128, W], f32, tag="vW")
            pooled = pp.tile([128, W], f32, tag="pooled")
            V.tensor_tensor(out=vW[:], in0=mDH[:, 0:W],
                            in1=mDH[:, 1:1 + W], op=Alu.max)
            V.tensor_tensor(out=pooled[:], in0=vW[:],
                            in1=mDH[:, 2:2 + W], op=Alu.max)
            mask = pp.tile([128, W], f32, tag="mask")
            V.tensor_tensor(out=mask[:], in0=xct[:, 1:1 + W],
                            in1=pooled[:], op=Alu.is_equal)

            ght = pp.tile([128, 1], f32, tag="ght")
            V.tensor_scalar(out=ght[:], in0=pid[:], scalar1=float(R),
                            scalar2=None, op0=Alu.add)

            for c in range(2):
                s = c * CK
                r = slice(0, 128)
                XCc = xct[r, 1 + s:1 + s + CK]
                XCp = xct[r, 2 + s:2 + s + CK]
                XCm = xct[r, 0 + s:0 + s + CK]
                XPc = xpt[r, 1 + s:1 + s + CK]
                XPp = xpt[r, 2 + s:2 + s + CK]
                XPm = xpt[r, 0 + s:0 + s + CK]
                XNc = xnt[r, 1 + s:1 + s + CK]
                XNp = xnt[r, 2 + s:2 + s + CK]
                XNm = xnt[r, 0 + s:0 + s + CK]
                maskc = mask[:, s:s + CK]

                # ---- TensorE: all derivative stencils -> PSUM ----
                xcd, xcu = tiles["cd"], tiles["cu"]
                xpd, xpu = tiles["pd"], tiles["pu"]
                xnd, xnu = tiles["nd"], tiles["nu"]
                byp = qp.tile([128, CK], f32, tag="byp")
                dyyp = qp.tile([128, CK], f32, tag="dyyp")
                dxyp = qp.tile([128, CK], f32, tag="dxyp")
                dysp = qp.tile([128, CK], f32, tag="dysp")
                T.matmul(byp[:], Ies[:], xcu[:, 1 + s:1 + s + CK],
                         start=True, stop=False)
                T.matmul(byp[:], In1s[:], xcd[:, 1 + s:1 + s + CK],
                         start=False, stop=True)
                T.matmul(dyyp[:], Ies[:], xcd[:, 1 + s:1 + s + CK],
                         start=True, stop=False)
                T.matmul(dyyp[:], Ies[:], xcu[:, 1 + s:1 + s + CK],
                         start=False, stop=False)
                T.matmul(dyyp[:], In2s[:], xct[:, 1 + s:1 + s + CK],
                         start=False, stop=True)
                T.matmul(dxyp[:], Ips[:], xcd[:, 0 + s:0 + s + CK],
                         start=True, stop=False)
                T.matmul(dxyp[:], Ins[:], xcd[:, 2 + s:2 + s + CK],
                         start=False, stop=False)
                T.matmul(dxyp[:], Ins[:], xcu[:, 0 + s:0 + s + CK],
                         start=False, stop=False)
                T.matmul(dxyp[:], Ips[:], xcu[:, 2 + s:2 + s + CK],
                         start=False, stop=True)
                T.matmul(dysp[:], Ips[:], xnd[:, 1 + s:1 + s + CK],
                         start=True, stop=False)
                T.matmul(dysp[:], Ins[:], xnu[:, 1 + s:1 + s + CK],
                         start=False, stop=False)
                T.matmul(dysp[:], Ins[:], xpd[:, 1 + s:1 + s + CK],
                         start=False, stop=False)
                T.matmul(dysp[:], Ips[:], xpu[:, 1 + s:1 + s + CK],
                         start=False, stop=True)
                dxxp = qp.tile([128, CK], f32, tag="dxxp")
                dssp = qp.tile([128, CK], f32, tag="dssp")
                dxsp = qp.tile([128, CK], f32, tag="dxsp")
                T.matmul(dxxp[:], Ies[:], xct[:, 2 + s:2 + s + CK],
                         start=True, stop=False)
                T.matmul(dxxp[:], Ies[:], xct[:, 0 + s:0 + s + CK],
                         start=False, stop=False)
                T.matmul(dxxp[:], In2s[:], xct[:, 1 + s:1 + s + CK],
                         start=False, stop=True)
                T.matmul(dssp[:], Ies[:], xpt[:, 1 + s:1 + s + CK],
                         start=True, stop=False)
                T.matmul(dssp[:], Ies[:], xnt[:, 1 + s:1 + s + CK],
                         start=False, stop=False)
                T.matmul(dssp[:], In2s[:], xct[:, 1 + s:1 + s + CK],
                         start=False, stop=True)
                T.matmul(dxsp[:], Ips[:], xnt[:, 0 + s:0 + s + CK],
                         start=True, stop=False)
                T.matmul(dxsp[:], Ins[:], xnt[:, 2 + s:2 + s + CK],
                         start=False, stop=False)
                T.matmul(dxsp[:], Ins[:], xpt[:, 0 + s:0 + s + CK],
                         start=False, stop=False)
                T.matmul(dxsp[:], Ips[:], xpt[:, 2 + s:2 + s + CK],
                         start=False, stop=True)

                # ---- ScalarE: PSUM evacuation with fused noise add ----
                A11 = kp.tile([128, CK], f32, tag="A11")
                A01 = kp.tile([128, CK], f32, tag="A01")
                A10 = kp.tile([128, CK], f32, tag="A10")
                A12 = kp.tile([128, CK], f32, tag="A12")
                A21 = kp.tile([128, CK], f32, tag="A21")
                bys = kp.tile([128, CK], f32, tag="bys")
                S.activation(A11[r, :], dyyp[r, :], Act.Identity, bias=nz(1, 1))
                S.activation(A01[r, :], dxyp[r, :], Act.Identity, bias=nz(0, 1))
                S.activation(A10[r, :], dxyp[r, :], Act.Identity, bias=nz(1, 0))
                S.activation(A12[r, :], dysp[r, :], Act.Identity, bias=nz(1, 2))
                S.activation(A21[r, :], dysp[r, :], Act.Identity, bias=nz(2, 1))
                S.copy(bys[r, :], byp[r, :])
                A00 = kp.tile([128, CK], f32, tag="A00")
                A22 = kp.tile([128, CK], f32, tag="A22")
                A02 = kp.tile([128, CK], f32, tag="A02")
                A20 = kp.tile([128, CK], f32, tag="A20")
                S.activation(A00[r, :], dxxp[r, :], Act.Identity, bias=nz(0, 0))
                S.activation(A22[r, :], dssp[r, :], Act.Identity, bias=nz(2, 2))
                S.activation(A02[r, :], dxsp[r, :], Act.Identity, bias=nz(0, 2))
                S.activation(A20[r, :], dxsp[r, :], Act.Identity, bias=nz(2, 0))

                # ---- VectorE: W/D-only derivatives ----
                bx = kp.tile([128, CK], f32, tag="bx")
                bz = kp.tile([128, CK], f32, tag="bz")
                V.tensor_tensor(out=bz[r, :], in0=XPc, in1=XNc, op=Alu.subtract)
                V.tensor_tensor(out=bx[r, :], in0=XCp, in1=XCm, op=Alu.subtract)

                # ---- adjugate (cofactor transpose), split across V and G ----
                adj = {}
                scr = [kp.tile([128, CK], f32, tag=f"scr{i}", name=f"scr{i}")
                       for i in range(6)]
                terms = [
                    # (key, p, q, c, d) -> adj = p*q - c*d   (engine alternates)
                    ("00", A11, A22, A12, A21),
                    ("01", A02, A21, A01, A22),
                    ("02", A01, A12, A02, A11),
                    ("10", A12, A20, A10, A22),
                    ("11", A00, A22, A02, A20),
                    ("12", A02, A10, A00, A12),
                    ("20", A10, A21, A11, A20),
                    ("21", A01, A20, A00, A21),
                    ("22", A00, A11, A01, A10),
                ]
                for i, (key, p, q, cc, dd) in enumerate(terms):
                    E = V
                    sa = scr[(2 * i) % 6]
                    sb = scr[(2 * i + 1) % 6]
                    a = kp.tile([128, CK], f32, tag=f"adj{key}")
                    E.tensor_tensor(out=sa[r, :], in0=p[r, :], in1=q[r, :],
                                    op=Alu.mult)
                    E.tensor_tensor(out=sb[r, :], in0=cc[r, :], in1=dd[r, :],
                                    op=Alu.mult)
                    E.tensor_tensor(out=a[r, :], in0=sa[r, :], in1=sb[r, :],
                                    op=Alu.subtract)
                    adj[key] = a

                # ---- det, masked reciprocal ----
                d1 = kp.tile([128, CK], f32, tag="d1")
                d2 = kp.tile([128, CK], f32, tag="d2")
                d3 = kp.tile([128, CK], f32, tag="d3")
                det = kp.tile([128, CK], f32, tag="det")
                V.tensor_tensor(out=d1[r, :], in0=A00[r, :], in1=adj["00"][r, :],
                                op=Alu.mult)
                V.tensor_tensor(out=d2[r, :], in0=A01[r, :], in1=adj["10"][r, :],
                                op=Alu.mult)
                V.tensor_tensor(out=d3[r, :], in0=A02[r, :], in1=adj["20"][r, :],
                                op=Alu.mult)
                V.tensor_tensor(out=d1[r, :], in0=d1[r, :], in1=d2[r, :],
                                op=Alu.add)
                V.tensor_tensor(out=det[r, :], in0=d1[r, :], in1=d3[r, :],
                                op=Alu.add)
                ds = kp.tile([128, CK], f32, tag="ds")
                G.memset(ds[r, :], 1.0)
                V.copy_predicated(ds[r, :], maskc.bitcast(mybir.dt.uint32),
                                  det[r, :])
                rdet = kp.tile([128, CK], f32, tag="rdet")
                V.reciprocal(rdet[r, :], ds[r, :])

                # ---- solution: sol_i = (adj_i0*bx + adj_i1*by + adj_i2*bz)*rdet
                sols = []
                for i, key in enumerate(("0", "1", "2")):
                    m1 = scr[0] if i != 0 else scr[3]
                    m2 = scr[1] if i != 0 else scr[4]
                    m3 = scr[2] if i != 0 else scr[5]
                    E1 = V
                    E2 = V
                    E1.tensor_tensor(out=m1[r, :], in0=adj[key + "0"][r, :],
                                     in1=bx[r, :], op=Alu.mult)
                    E2.tensor_tensor(out=m2[r, :], in0=adj[key + "1"][r, :],
                                     in1=bys[r, :], op=Alu.mult)
                    E1.tensor_tensor(out=m3[r, :], in0=adj[key + "2"][r, :],
                                     in1=bz[r, :], op=Alu.mult)
                    E2.tensor_tensor(out=m1[r, :], in0=m1[r, :], in1=m2[r, :],
                                     op=Alu.add)
                    E1.tensor_tensor(out=m1[r, :], in0=m1[r, :], in1=m3[r, :],
                                     op=Alu.add)
                    so = kp.tile([128, CK], f32, tag=f"sol{key}")
                    E2.tensor_tensor(out=so[r, :], in0=m1[r, :], in1=rdet[r, :],
                                     op=Alu.mult)
                    sols.append(so)

                # ---- discard-big + mask fold: keep = -0.5*mask*(|sol|<=1.4) --
                ab0 = kp.tile([128, CK], f32, tag="ab0")
                ab1 = kp.tile([128, CK], f32, tag="ab1")
                ab2 = kp.tile([128, CK], f32, tag="ab2")
                S.activation(ab0[r, :], sols[0][r, :], Act.Abs)
                S.activation(ab1[r, :], sols[1][r, :], Act.Abs)
                S.activation(ab2[r, :], sols[2][r, :], Act.Abs)
                am1 = kp.tile([128, CK], f32, tag="am1")
                am2 = kp.tile([128, CK], f32, tag="am2")
                V.tensor_tensor(out=am1[r, :], in0=ab0[r, :],
                                in1=ab1[r, :], op=Alu.max)
                V.tensor_tensor(out=am2[r, :], in0=am1[r, :], in1=ab2[r, :],
                                op=Alu.max)
                nb = kp.tile([128, CK], f32, tag="nb")
                G.tensor_scalar(out=nb[r, :], in0=am2[r, :], scalar1=1.4,
                                scalar2=-0.5, op0=Alu.is_le, op1=Alu.mult)
                keep = kp.tile([128, CK], f32, tag="keep")
                V.tensor_tensor(out=keep[r, :], in0=nb[r, :], in1=maskc,
                                op=Alu.mult)
                t0 = kp.tile([128, CK], f32, tag="t0")
                t1 = kp.tile([128, CK], f32, tag="t1")
                t2 = kp.tile([128, CK], f32, tag="t2")
                V.tensor_tensor(out=t0[r, :], in0=sols[0][r, :], in1=keep[r, :],
                                op=Alu.mult)
                V.tensor_tensor(out=t1[r, :], in0=sols[1][r, :], in1=keep[r, :],
                                op=Alu.mult)
                V.tensor_tensor(out=t2[r, :], in0=sols[2][r, :], in1=keep[r, :],
                                op=Alu.mult)

                # ---- outputs ----
                cdt = op_.tile([128, CK], f32, tag="cdt")
                cht = op_.tile([128, CK], f32, tag="cht")
                cwt = op_.tile([128, CK], f32, tag="cwt")
                G.tensor_scalar(out=cdt[r, :], in0=t2[r, :], scalar1=dcol,
                                scalar2=None, op0=Alu.add)
                G.tensor_scalar(out=cht[r, :], in0=t1[r, :],
                                scalar1=ght[:, 0:1], scalar2=None,
                                op0=Alu.add)
                V.tensor_tensor(out=cwt[r, :], in0=t0[r, :],
                                in1=gws[r, s:s + CK], op=Alu.add)
                w1 = scr[0]
                w2 = scr[1]
                w3 = scr[2]
                V.tensor_tensor(out=w1[r, :], in0=bx[r, :], in1=t0[r, :],
                                op=Alu.mult)
                V.tensor_tensor(out=w2[r, :], in0=bys[r, :], in1=t1[r, :],
                                op=Alu.mult)
                V.tensor_tensor(out=w3[r, :], in0=bz[r, :], in1=t2[r, :],
                                op=Alu.mult)
                V.tensor_tensor(out=w1[r, :], in0=w1[r, :], in1=w2[r, :],
                                op=Alu.add)
                V.tensor_tensor(out=w1[r, :], in0=w1[r, :], in1=w3[r, :],
                                op=Alu.add)
                y1 = kp.tile([128, CK], f32, tag="y1")
                y2 = op_.tile([128, CK], f32, tag="y2")
                V.scalar_tensor_tensor(out=y1[r, :], in0=w1[r, :], scalar=0.25,
                                       in1=XCc, op0=Alu.mult, op1=Alu.add)
                V.scalar_tensor_tensor(out=y2[r, :], in0=maskc, scalar=10.0,
                                       in1=y1[r, :], op0=Alu.mult, op1=Alu.add)

                nc.sync.dma_start(out=y_d[R:R + 128, s:s + CK], in_=y2[:, :])
                nc.sync.dma_start(out=cd_d[R:R + 128, s:s + CK], in_=cdt[:, :])
                nc.sync.dma_start(out=ch_d[R:R + 128, s:s + CK], in_=cht[:, :])
                nc.sync.dma_start(out=cw_d[R:R + 128, s:s + CK], in_=cwt[:, :])

    nc.compile()
    return nc


def _get_nc(reps=1):
    key = f"nc{reps}"
    if key not in _CACHE:
        _CACHE[key] = _build_nc(reps)
    return _CACHE[key]


def make_core_inputs(x, hes_noise):
    """Host-side sharding: per-core input dicts."""
    x = np.asarray(x)
    hes_noise = np.asarray(hes_noise, dtype=np.float32)
    B, C, D = x.shape[0], x.shape[1], x.shape[2]
    ins = []
    for c in range(NCORES):
        b, d = c // D, c % D
        cst = np.zeros((128, 16), np.float32)
        cst[:, 0:9] = (hes_noise.reshape(-1) * np.float32(EPS))[None, :]
        cst[:, 9] = np.float32(d)
        ins.append({
            "xp": np.ascontiguousarray(x[b, 0, max(d - 1, 0)]),
            "xc": np.ascontiguousarray(x[b, 0, d]),
            "xn": np.ascontiguousarray(x[b, 0, min(d + 1, D - 1)]),
            "consts": cst,
        })
    return ins


def assemble_outputs(results, B=2, C=1, D=4):
    coords = np.empty((B, C, 3, D, H, W), np.float32)
    y = np.empty((B, C, D, H, W), np.float32)
    for c in range(NCORES):
        b, d = c // D, c % D
        coords[b, 0, 0, d] = results[c]["cd"]
        coords[b, 0, 1, d] = results[c]["ch"]
        coords[b, 0, 2, d] = results[c]["cw"]
        y[b, 0, d] = results[c]["y"]
    return coords, y


def kernel(x, hes_noise):
    nc = _get_nc()
    ins = make_core_inputs(x, hes_noise)
    res = run_bass_kernel_spmd(nc, ins, core_ids=list(range(NCORES)))
    return assemble_outputs(res.results)


# revision 15
# speedup vs baseline: 26.4570x; 1.0159x over previous
"""Trainium2 Bass kernel for ConvQuadInterp3d (3D NMS + quadratic refinement).

Sharding: fully data-parallel. Core c handles plane (b=c//4, d=c%4) of the
(2,1,4,1024,1024) input. Host passes each core its own plane plus the
depth-clamped prev/next planes, so one SPMD program serves all 8 cores.

Per-core kernel (1024x1024 plane, fp32):
  - 8 row-tiles of exactly 128 output rows. Each plane is loaded at three
    row alignments (rows R-1.., R.., R+1..) so every H-neighbor tap is plain
    data on matching partitions - no partition shifts anywhere (edge rows
    replicated during the DMA load; W edges via padded [128,1026] tiles).
  - 3x3x3 strict NMS max-pool computed separably (H across the three
    alignments, then D, then W via shifted views) - exact, so the
    x == pooled equality mask matches the reference bit-for-bit.
  - ALL nine derivative stencils run on TensorE as matmuls with +/-scaled
    identity stationaries over the shifted-alignment tiles, accumulating in
    PSUM; hes_noise*EPS is added during the PSUM->SBUF evacuation on
    ScalarE (activation bias), per-partition scalars supplied by the host.
  - 3x3 solve via the adjugate/Cramer rule; det is replaced by 1.0 outside
    the NMS mask (memset + copy_predicated over a uint32-bitcast mask) so
    the reciprocal stays finite.
  - offsets dx = -0.5*sol' (sol' solves with unscaled 2x gradients), with
    the |dx|>0.7 discard folded into one keep multiplier.

Measured (8-core SPMD, axon trn2): relative error vs reference 6.0e-08;
device time per pass ~0.92-1.1 ms (reps-slope method; wall-clock per call
is dominated by ~20-30 ms of axon-relay I/O streaming). VectorE is the
wall: this walrus build rejects GpSimd TensorTensor (Pool-engine check),
so the ~1200 2-input elementwise ops (pool maxes + cofactors/solve)
serialize on DVE. TensorE carries all linear stencils (23 matmuls/chunk),
ScalarE the PSUM evacuations + noise adds + abs, GpSimd the memsets,
tensor_scalar ops and edge-pad copies.
"""
import numpy as np
import concourse.bass as bass
import concourse.mybir as mybir
from concourse import tile, bacc
from concourse.bass_utils import run_bass_kernel_spmd

f32 = mybir.dt.float32
Alu = mybir.AluOpType
Act = mybir.ActivationFunctionType

EPS = 1e-7
H = 1024
W = 1024
PW = W + 2
NT = 8           # row tiles
RPT = 126        # valid output rows per tile
CK = 512         # chunk width for the solve phase
NCORES = 8

_CACHE: dict = {}


def _build_nc(reps=1):
    nc = bacc.Bacc("TRN2", target_bir_lowering=False, debug=False,
                   num_devices=NCORES)
    xp_d = nc.dram_tensor("xp", [H, W], f32, kind="ExternalInput")
    xc_d = nc.dram_tensor("xc", [H, W], f32, kind="ExternalInput")
    xn_d = nc.dram_tensor("xn", [H, W], f32, kind="ExternalInput")
    cst_d = nc.dram_tensor("consts", [128, 16], f32, kind="ExternalInput")
    y_d = nc.dram_tensor("y", [H, W], f32, kind="ExternalOutput")
    cd_d = nc.dram_tensor("cd", [H, W], f32, kind="ExternalOutput")
    ch_d = nc.dram_tensor("ch", [H, W], f32, kind="ExternalOutput")
    cw_d = nc.dram_tensor("cw", [H, W], f32, kind="ExternalOutput")

    # shift matrices (lhsT layout: out[m] = sum_k L[k,m] * in[k])
    Lp = np.zeros((128, 128), np.float32)
    Lp[np.arange(1, 128), np.arange(127)] = 1.0      # out[m] = in[m+1]
    Lm = np.zeros((128, 128), np.float32)
    Lm[np.arange(127), np.arange(1, 128)] = 1.0      # out[m] = in[m-1]
    I = np.eye(128, dtype=np.float32)
    L1_np = Lp - Lm                                  # by' (row grad, x2)
    L2_np = Lp + Lm - 2.0 * I                        # dyy
    L3_np = (0.25 * (Lp - Lm)).astype(np.float32)    # dxy/dys +
    L4_np = (-L3_np).astype(np.float32)              # dxy/dys -
    Ieye_np = I.copy()
    In1_np = (-1.0 * I).astype(np.float32)
    In2_np = (-2.0 * I).astype(np.float32)
    Ip025_np = (0.25 * I).astype(np.float32)
    In025_np = (-0.25 * I).astype(np.float32)
    gw_np = np.broadcast_to(np.arange(W, dtype=np.float32), (128, W)).copy()
    gw_d = nc.inline_tensor(gw_np, "gwi")
    pid_np = np.arange(128, dtype=np.float32).reshape(128, 1).copy()
    pid_d = nc.inline_tensor(pid_np, "pidx")
    ie_d = nc.inline_tensor(Ieye_np, "Ieye")
    in1_d = nc.inline_tensor(In1_np, "In1")
    in2_d = nc.inline_tensor(In2_np, "In2")
    ip_d = nc.inline_tensor(Ip025_np, "Ip025")
    in_d = nc.inline_tensor(In025_np, "In025")

    V = nc.vector
    G = nc.gpsimd
    S = nc.scalar
    T = nc.tensor

    with tile.TileContext(nc) as tc, \
         tc.tile_pool(name="cst", bufs=1) as cp, \
         tc.tile_pool(name="inp", bufs=2) as ip, \
         tc.tile_pool(name="pool", bufs=1) as pp, \
         tc.tile_pool(name="ck", bufs=1) as kp, \
         tc.tile_pool(name="outp", bufs=2) as op_, \
         tc.tile_pool(name="psum", bufs=1, space="PSUM") as qp:

        gws = cp.tile([128, W], f32, tag="gws")
        cst = cp.tile([128, 16], f32, tag="cstt")
        pid = cp.tile([128, 1], f32, tag="pidt")
        Ies = cp.tile([128, 128], f32, tag="Ies")
        In1s = cp.tile([128, 128], f32, tag="In1s")
        In2s = cp.tile([128, 128], f32, tag="In2s")
        Ips = cp.tile([128, 128], f32, tag="Ips")
        Ins = cp.tile([128, 128], f32, tag="Ins")
        nc.sync.dma_start(out=gws[:], in_=gw_d[:])
        nc.sync.dma_start(out=cst[:], in_=cst_d[:])
        nc.sync.dma_start(out=pid[:], in_=pid_d[:])
        nc.sync.dma_start(out=Ies[:], in_=ie_d[:])
        nc.sync.dma_start(out=In1s[:], in_=in1_d[:])
        nc.sync.dma_start(out=In2s[:], in_=in2_d[:])
        nc.sync.dma_start(out=Ips[:], in_=ip_d[:])
        nc.sync.dma_start(out=Ins[:], in_=in_d[:])

        def nz(i, j):                       # hes_noise[i,j]*EPS per-partition col
            return cst[:, 3 * i + j:3 * i + j + 1]

        dcol = cst[:, 9:10]             # depth value

        for rep in range(reps):
         for rt in range(NT):
            R = rt * 128

            tiles = {}
            for pl, src_d in (("p", xp_d), ("c", xc_d), ("n", xn_d)):
                for al in ("d", "c", "u"):
                    bufs = 2 if al == "c" else 1
                    t = ip.tile([128, PW], f32, tag=f"x{pl}{al}",
                                name=f"x{pl}{al}", bufs=bufs)
                    if al == "c":
                        nc.sync.dma_start(out=t[:, 1:1 + W],
                                          in_=src_d[R:R + 128, :])
                    elif al == "d":
                        if rt == 0:
                            nc.sync.dma_start(out=t[1:128, 1:1 + W],
                                              in_=src_d[0:127, :])
                            nc.sync.dma_start(out=t[0:1, 1:1 + W],
                                              in_=src_d[0:1, :])
                        else:
                            nc.sync.dma_start(out=t[:, 1:1 + W],
                                              in_=src_d[R - 1:R + 127, :])
                    else:
                        if rt == NT - 1:
                            nc.sync.dma_start(out=t[0:127, 1:1 + W],
                                              in_=src_d[R + 1:H, :])
                            nc.sync.dma_start(out=t[127:128, 1:1 + W],
                                              in_=src_d[H - 1:H, :])
                        else:
                            nc.sync.dma_start(out=t[:, 1:1 + W],
                                              in_=src_d[R + 1:R + 129, :])
                    G.tensor_copy(out=t[:, 0:1], in_=t[:, 1:2])
                    G.tensor_copy(out=t[:, PW - 1:PW],
                                  in_=t[:, PW - 2:PW - 1])
                    tiles[pl + al] = t
            xpt, xct, xnt = tiles["pc"], tiles["cc"], tiles["nc"]

            # ---- 3x3x3 max pool (separable, H-first across alignments) ----
            mH = {}
            for pl in ("p", "c", "n"):
                tF = pp.tile([128, PW], f32, tag="tF", name="tF")
                m = pp.tile([128, PW], f32, tag=f"mH{pl}", name=f"mH{pl}")
                V.tensor_tensor(out=tF[:], in0=tiles[pl + "d"][:],
                                in1=tiles[pl + "c"][:], op=Alu.max)
                V.tensor_tensor(out=m[:], in0=tF[:], in1=tiles[pl + "u"][:],
                                op=Alu.max)
                mH[pl] = m
            uD = pp.tile([128, PW], f32, tag="uD")
            mDH = pp.tile([128, PW], f32, tag="mDH")
            V.tensor_tensor(out=uD[:], in0=mH["p"][:], in1=mH["c"][:],
                            op=Alu.max)
            V.tensor_tensor(out=mDH[:], in0=uD[:], in1=mH["n"][:], op=Alu.max)
            vW = pp.tile([128, W], f32, tag="vW")
            pooled = pp.tile([128, W], f32, tag="pooled")
            V.tensor_tensor(out=vW[:], in0=mDH[:, 0:W],
                            in1=mDH[:, 1:1 + W], op=Alu.max)
            V.tensor_tensor(out=pooled[:], in0=vW[:],
                            in1=mDH[:, 2:2 + W], op=Alu.max)
            mask = pp.tile([128, W], f32, tag="mask")
            V.tensor_tensor(out=mask[:], in0=xct[:, 1:1 + W],
                            in1=pooled[:], op=Alu.is_equal)

            ght = pp.tile([128, 1], f32, tag="ght")
            G.tensor_scalar(out=ght[:], in0=pid[:], scalar1=float(R),
                            scalar2=None, op0=Alu.add)

            for c in range(2):
                s = c * CK
                r = slice(0, 128)
                XCc = xct[r, 1 + s:1 + s + CK]
                XCp = xct[r, 2 + s:2 + s + CK]
                XCm = xct[r, 0 + s:0 + s + CK]
                XPc = xpt[r, 1 + s:1 + s + CK]
                XPp = xpt[r, 2 + s:2 + s + CK]
                XPm = xpt[r, 0 + s:0 + s + CK]
                XNc = xnt[r, 1 + s:1 + s + CK]
                XNp = xnt[r, 2 + s:2 + s + CK]
                XNm = xnt[r, 0 + s:0 + s + CK]
                maskc = mask[:, s:s + CK]

                # ---- TensorE: all derivative stencils -> PSUM ----
                xcd, xcu = tiles["cd"], tiles["cu"]
                xpd, xpu = tiles["pd"], tiles["pu"]
                xnd, xnu = tiles["nd"], tiles["nu"]
                byp = qp.tile([128, CK], f32, tag="byp")
                dyyp = qp.tile([128, CK], f32, tag="dyyp", bufs=2)
                dxyp = qp.tile([128, CK], f32, tag="dxyp")
                dysp = qp.tile([128, CK], f32, tag="dysp")
                T.matmul(byp[:], Ies[:], xcu[:, 1 + s:1 + s + CK],
                         start=True, stop=False)
                T.matmul(byp[:], In1s[:], xcd[:, 1 + s:1 + s + CK],
                         start=False, stop=True)
                T.matmul(dyyp[:], Ies[:], xcd[:, 1 + s:1 + s + CK],
                         start=True, stop=False)
                T.matmul(dyyp[:], Ies[:], xcu[:, 1 + s:1 + s + CK],
                         start=False, stop=False)
                T.matmul(dyyp[:], In2s[:], xct[:, 1 + s:1 + s + CK],
                         start=False, stop=True)
                T.matmul(dxyp[:], Ips[:], xcd[:, 0 + s:0 + s + CK],
                         start=True, stop=False)
                T.matmul(dxyp[:], Ins[:], xcd[:, 2 + s:2 + s + CK],
                         start=False, stop=False)
                T.matmul(dxyp[:], Ins[:], xcu[:, 0 + s:0 + s + CK],
                         start=False, stop=False)
                T.matmul(dxyp[:], Ips[:], xcu[:, 2 + s:2 + s + CK],
                         start=False, stop=True)
                T.matmul(dysp[:], Ips[:], xnd[:, 1 + s:1 + s + CK],
                         start=True, stop=False)
                T.matmul(dysp[:], Ins[:], xnu[:, 1 + s:1 + s + CK],
                         start=False, stop=False)
                T.matmul(dysp[:], Ins[:], xpd[:, 1 + s:1 + s + CK],
                         start=False, stop=False)
                T.matmul(dysp[:], Ips[:], xpu[:, 1 + s:1 + s + CK],
                         start=False, stop=True)
                dxxp = qp.tile([128, CK], f32, tag="dxxp")
                dssp = qp.tile([128, CK], f32, tag="dssp")
                dxsp = qp.tile([128, CK], f32, tag="dxsp")
                T.matmul(dxxp[:], Ies[:], xct[:, 2 + s:2 + s + CK],
                         start=True, stop=False)
                T.matmul(dxxp[:], Ies[:], xct[:, 0 + s:0 + s + CK],
                         start=False, stop=False)
                T.matmul(dxxp[:], In2s[:], xct[:, 1 + s:1 + s + CK],
                         start=False, stop=True)
                T.matmul(dssp[:], Ies[:], xpt[:, 1 + s:1 + s + CK],
                         start=True, stop=False)
                T.matmul(dssp[:], Ies[:], xnt[:, 1 + s:1 + s + CK],
                         start=False, stop=False)
                T.matmul(dssp[:], In2s[:], xct[:, 1 + s:1 + s + CK],
                         start=False, stop=True)
                T.matmul(dxsp[:], Ips[:], xnt[:, 0 + s:0 + s + CK],
                         start=True, stop=False)
                T.matmul(dxsp[:], Ins[:], xnt[:, 2 + s:2 + s + CK],
                         start=False, stop=False)
                T.matmul(dxsp[:], Ins[:], xpt[:, 0 + s:0 + s + CK],
                         start=False, stop=False)
                T.matmul(dxsp[:], Ips[:], xpt[:, 2 + s:2 + s + CK],
                         start=False, stop=True)

                # ---- ScalarE: PSUM evacuation with fused noise add ----
                A11 = kp.tile([128, CK], f32, tag="A11")
                A01 = kp.tile([128, CK], f32, tag="A01")
                A10 = kp.tile([128, CK], f32, tag="A10")
                A12 = kp.tile([128, CK], f32, tag="A12")
                A21 = kp.tile([128, CK], f32, tag="A21")
                bys = kp.tile([128, CK], f32, tag="bys")
                S.activation(A11[r, :], dyyp[r, :], Act.Identity, bias=nz(1, 1))
                S.activation(A01[r, :], dxyp[r, :], Act.Identity, bias=nz(0, 1))
                S.activation(A10[r, :], dxyp[r, :], Act.Identity, bias=nz(1, 0))
                S.activation(A12[r, :], dysp[r, :], Act.Identity, bias=nz(1, 2))
                S.activation(A21[r, :], dysp[r, :], Act.Identity, bias=nz(2, 1))
                S.copy(bys[r, :], byp[r, :])
                A00 = kp.tile([128, CK], f32, tag="A00")
                A22 = kp.tile([128, CK], f32, tag="A22")
                A02 = kp.tile([128, CK], f32, tag="A02")
                A20 = kp.tile([128, CK], f32, tag="A20")
                S.activation(A00[r, :], dxxp[r, :], Act.Identity, bias=nz(0, 0))
                S.activation(A22[r, :], dssp[r, :], Act.Identity, bias=nz(2, 2))
                S.activation(A02[r, :], dxsp[r, :], Act.Identity, bias=nz(0, 2))
                S.activation(A20[r, :], dxsp[r, :], Act.Identity, bias=nz(2, 0))

                # ---- VectorE: W/D-only derivatives ----
                bx = kp.tile([128, CK], f32, tag="bx")
                bz = kp.tile([128, CK], f32, tag="bz")
                V.tensor_tensor(out=bz[r, :], in0=XPc, in1=XNc, op=Alu.subtract)
                V.tensor_tensor(out=bx[r, :], in0=XCp, in1=XCm, op=Alu.subtract)

                # ---- adjugate (cofactor transpose), split across V and G ----
                adj = {}
                scr = [kp.tile([128, CK], f32, tag=f"scr{i}", name=f"scr{i}")
                       for i in range(6)]
                terms = [
                    # (key, p, q, c, d) -> adj = p*q - c*d   (engine alternates)
                    ("00", A11, A22, A12, A21),
                    ("01", A02, A21, A01, A22),
                    ("02", A01, A12, A02, A11),
                    ("10", A12, A20, A10, A22),
                    ("11", A00, A22, A02, A20),
                    ("12", A02, A10, A00, A12),
                    ("20", A10, A21, A11, A20),
                    ("21", A01, A20, A00, A21),
                    ("22", A00, A11, A01, A10),
                ]
                for i, (key, p, q, cc, dd) in enumerate(terms):
                    E = V
                    sa = scr[(2 * i) % 6]
                    sb = scr[(2 * i + 1) % 6]
                    a = kp.tile([128, CK], f32, tag=f"adj{key}")
                    E.tensor_tensor(out=sa[r, :], in0=p[r, :], in1=q[r, :],
                                    op=Alu.mult)
                    E.tensor_tensor(out=sb[r, :], in0=cc[r, :], in1=dd[r, :],
                                    op=Alu.mult)
                    E.tensor_tensor(out=a[r, :], in0=sa[r, :], in1=sb[r, :],
                                    op=Alu.subtract)
                    adj[key] = a

                # ---- det, masked reciprocal ----
                d1 = kp.tile([128, CK], f32, tag="d1")
                d2 = kp.tile([128, CK], f32, tag="d2")
                d3 = kp.tile([128, CK], f32, tag="d3")
                det = kp.tile([128, CK], f32, tag="det")
                V.tensor_tensor(out=d1[r, :], in0=A00[r, :], in1=adj["00"][r, :],
                                op=Alu.mult)
                V.tensor_tensor(out=d2[r, :], in0=A01[r, :], in1=adj["10"][r, :],
                                op=Alu.mult)
                V.tensor_tensor(out=d3[r, :], in0=A02[r, :], in1=adj["20"][r, :],
                                op=Alu.mult)
                V.tensor_tensor(out=d1[r, :], in0=d1[r, :], in1=d2[r, :],
                                op=Alu.add)
                V.tensor_tensor(out=det[r, :], in0=d1[r, :], in1=d3[r, :],
                                op=Alu.add)
                ds = kp.tile([128, CK], f32, tag="ds")
                G.memset(ds[r, :], 1.0)
                V.copy_predicated(ds[r, :], maskc.bitcast(mybir.dt.uint32),
                                  det[r, :])
                rdet = kp.tile([128, CK], f32, tag="rdet")
                V.reciprocal(rdet[r, :], ds[r, :])

                # ---- solution: sol_i = (adj_i0*bx + adj_i1*by + adj_i2*bz)*rdet
                sols = []
                for i, key in enumerate(("0", "1", "2")):
                    m1 = scr[0] if i != 0 else scr[3]
                    m2 = scr[1] if i != 0 else scr[4]
                    m3 = scr[2] if i != 0 else scr[5]
                    E1 = V
                    E2 = V
                    E1.tensor_tensor(out=m1[r, :], in0=adj[key + "0"][r, :],
                                     in1=bx[r, :], op=Alu.mult)
                    E2.tensor_tensor(out=m2[r, :], in0=adj[key + "1"][r, :],
                                     in1=bys[r, :], op=Alu.mult)
                    E1.tensor_tensor(out=m3[r, :], in0=adj[key + "2"][r, :],
                                     in1=bz[r, :], op=Alu.mult)
                    E2.tensor_tensor(out=m1[r, :], in0=m1[r, :], in1=m2[r, :],
                                     op=Alu.add)
                    E1.tensor_tensor(out=m1[r, :], in0=m1[r, :], in1=m3[r, :],
                                     op=Alu.add)
                    so = kp.tile([128, CK], f32, tag=f"sol{key}")
                    E2.tensor_tensor(out=so[r, :], in0=m1[r, :], in1=rdet[r, :],
                                     op=Alu.mult)
                    sols.append(so)

                # ---- discard-big + mask fold: keep = -0.5*mask*(|sol|<=1.4) --
                ab0 = kp.tile([128, CK], f32, tag="ab0")
                ab1 = kp.tile([128, CK], f32, tag="ab1")
                ab2 = kp.tile([128, CK], f32, tag="ab2")
                S.activation(ab0[r, :], sols[0][r, :], Act.Abs)
                S.activation(ab1[r, :], sols[1][r, :], Act.Abs)
                S.activation(ab2[r, :], sols[2][r, :], Act.Abs)
                am1 = kp.tile([128, CK], f32, tag="am1")
                am2 = kp.tile([128, CK], f32, tag="am2")
                V.tensor_tensor(out=am1[r, :], in0=ab0[r, :],
                                in1=ab1[r, :], op=Alu.max)
                V.tensor_tensor(out=am2[r, :], in0=am1[r, :], in1=ab2[r, :],
                                op=Alu.max)
                nb = kp.tile([128, CK], f32, tag="nb")
                G.tensor_scalar(out=nb[r, :], in0=am2[r, :], scalar1=1.4,
                                scalar2=-0.5, op0=Alu.is_le, op1=Alu.mult)
                keep = kp.tile([128, CK], f32, tag="keep")
                V.tensor_tensor(out=keep[r, :], in0=nb[r, :], in1=maskc,
                                op=Alu.mult)
                t0 = kp.tile([128, CK], f32, tag="t0")
                t1 = kp.tile([128, CK], f32, tag="t1")
                t2 = kp.tile([128, CK], f32, tag="t2")
                V.tensor_tensor(out=t0[r, :], in0=sols[0][r, :], in1=keep[r, :],
                                op=Alu.mult)
                V.tensor_tensor(out=t1[r, :], in0=sols[1][r, :], in1=keep[r, :],
                                op=Alu.mult)
                V.tensor_tensor(out=t2[r, :], in0=sols[2][r, :], in1=keep[r, :],
                                op=Alu.mult)

                # ---- outputs ----
                cdt = op_.tile([128, CK], f32, tag="cdt")
                cht = op_.tile([128, CK], f32, tag="cht")
                cwt = op_.tile([128, CK], f32, tag="cwt")
                G.tensor_scalar(out=cdt[r, :], in0=t2[r, :], scalar1=dcol,
                                scalar2=None, op0=Alu.add)
                G.tensor_scalar(out=cht[r, :], in0=t1[r, :],
                                scalar1=ght[:, 0:1], scalar2=None,
                                op0=Alu.add)
                V.tensor_tensor(out=cwt[r, :], in0=t0[r, :],
                                in1=gws[r, s:s + CK], op=Alu.add)
                w1 = scr[0]
                w2 = scr[1]
                w3 = scr[2]
                V.tensor_tensor(out=w1[r, :], in0=bx[r, :], in1=t0[r, :],
                                op=Alu.mult)
                V.tensor_tensor(out=w2[r, :], in0=bys[r, :], in1=t1[r, :],
                                op=Alu.mult)
                V.tensor_tensor(out=w3[r, :], in0=bz[r, :], in1=t2[r, :],
                                op=Alu.mult)
                V.tensor_tensor(out=w1[r, :], in0=w1[r, :], in1=w2[r, :],
                                op=Alu.add)
                V.tensor_tensor(out=w1[r, :], in0=w1[r, :], in1=w3[r, :],
                                op=Alu.add)
                y1 = kp.tile([128, CK], f32, tag="y1")
                y2 = op_.tile([128, CK], f32, tag="y2")
                V.scalar_tensor_tensor(out=y1[r, :], in0=w1[r, :], scalar=0.25,
                                       in1=XCc, op0=Alu.mult, op1=Alu.add)
                V.scalar_tensor_tensor(out=y2[r, :], in0=maskc, scalar=10.0,
                                       in1=y1[r, :], op0=Alu.mult, op1=Alu.add)

                nc.sync.dma_start(out=y_d[R:R + 128, s:s + CK], in_=y2[:, :])
                nc.sync.dma_start(out=cd_d[R:R + 128, s:s + CK], in_=cdt[:, :])
                nc.sync.dma_start(out=ch_d[R:R + 128, s:s + CK], in_=cht[:, :])
                nc.sync.dma_start(out=cw_d[R:R + 128, s:s + CK], in_=cwt[:, :])

    nc.compile()
    return nc


def _get_nc(reps=1):
    key = f"nc{reps}"
    if key not in _CACHE:
        _CACHE[key] = _build_nc(reps)
    return _CACHE[key]


def make_core_inputs(x, hes_noise):
    """Host-side sharding: per-core input dicts."""
    x = np.asarray(x)
    hes_noise = np.asarray(hes_noise, dtype=np.float32)
    B, C, D = x.shape[0], x.shape[1], x.shape[2]
    ins = []
    for c in range(NCORES):
        b, d = c // D, c % D
        cst = np.zeros((128, 16), np.float32)
        cst[:, 0:9] = (hes_noise.reshape(-1) * np.float32(EPS))[None, :]
        cst[:, 9] = np.float32(d)
        ins.append({
            "xp": np.ascontiguousarray(x[b, 0, max(d - 1, 0)]),
            "xc": np.ascontiguousarray(x[b, 0, d]),
            "xn": np.ascontiguousarray(x[b, 0, min(d + 1, D - 1)]),
            "consts": cst,
        })
    return ins


def assemble_outputs(results, B=2, C=1, D=4):
    coords = np.empty((B, C, 3, D, H, W), np.float32)
    y = np.empty((B, C, D, H, W), np.float32)
    for c in range(NCORES):
        b, d = c // D, c % D
        coords[b, 0, 0, d] = results[c]["cd"]
        coords[b, 0, 1, d] = results[c]["ch"]
        coords[b, 0, 2, d] = results[c]["cw"]
        y[b, 0, d] = results[c]["y"]
    return coords, y


def kernel(x, hes_noise):
    nc = _get_nc()
    ins = make_core_inputs(x, hes_noise)
    res = run_bass_kernel_spmd(nc, ins, core_ids=list(range(NCORES)))
    return assemble_outputs(res.results)
